# revision 2
# baseline (speedup 1.0000x reference)
"""GSA layer (Gaussian-biased axial attention) Trainium2 Bass kernel.

Full inputs in, full output out. Shards batch B=8 across 8 NeuronCores
(data parallel, one image per core). Self-contained: hardcodes shapes.

v2: all-bf16 matmul pipeline (fp32 matmuls are 4 cycles/row on TRN2,
bf16 is 1). PSUM accumulation and softmax stay fp32. bf16 intermediates
in DRAM halve pass-B DMA traffic.

Per-core dataflow (image = 64x64 tokens, D=1024):
  pass A (row):  stream xT chunks (512 tokens = 8 image rows):
                 Q,K proj ([e,t] layout), V proj ([t,e] layout),
                 fused row attention (scores bf16 paired matmuls, softmax
                 fp32 on free axis, DVE block transposes, bf16 AV),
                 writes to DRAM: V natural, row-major Q/K, r_outT.
  pass B (col):  per 8-column chunk: col attention (mirrored),
                 add r_outT, fused output projection -> outT col-order.
Host: transposes/reshapes, folds bv into output bias, unshards output.
"""

import os
import numpy as np
import ml_dtypes

import concourse.bass as bass
import concourse.mybir as mybir
import concourse.tile as tile
from concourse import bacc
from concourse import bass_utils

F32 = mybir.dt.float32
F32R = mybir.dt.float32r
BF16 = mybir.dt.bfloat16
AX = mybir.AxisListType
ALU = mybir.AluOpType
ACTF = mybir.ActivationFunctionType

B, H, W, D = 8, 64, 64, 1024
P = 128
HW = H * W            # 4096 tokens per image
CH = 512              # token chunk (8 image rows / 8 image cols)
NCH = HW // CH        # 8 chunks
EO = D // P           # 8 partition tiles of the 1024 dim

_cache = {}


def _softmax_block(nc, pool, half_aps, bm_sb, pnT):
    """Softmax over the free axis of two [64,64] PSUM score blocks
    (half_aps[hl], partition base hl*64), then write transposed bf16
    weights into pnT [128, 64] (half hl at rows hl*64:hl*64+64)."""
    sc = pool.tile([P, 64], F32, tag="sm_sc")
    negm = pool.tile([P, 1], F32, tag="sm_negm")
    ssum = pool.tile([P, 1], F32, tag="sm_ssum")
    rinv = pool.tile([P, 1], F32, tag="sm_rinv")
    pn = pool.tile([P, 64], F32, tag="sm_pn")
    pnn = pool.tile([P, 64], BF16, tag="sm_pnn")
    for hl in range(2):
        blk = slice(hl * 64, hl * 64 + 64)
        nc.vector.tensor_tensor(sc[blk, :], half_aps[hl],
                                bm_sb[blk, :], ALU.add)
        nc.vector.tensor_reduce(negm[blk, :], sc[blk, :], axis=AX.X,
                                op=ALU.max, negate=True)
        nc.scalar.activation(pn[blk, :], sc[blk, :], ACTF.Exp,
                             bias=negm[blk, 0:1], accum_out=ssum[blk, 0:1])
    nc.vector.reciprocal(rinv[:], ssum[:])
    nc.vector.tensor_scalar_mul(pnn[:], pn[:], rinv[:, 0:1])
    # transpose each 64x64 half via 4 DVE 32x32 block transposes
    for hl in range(2):
        o = hl * 64
        for bi in range(2):
            for bj in range(2):
                nc.vector.transpose(
                    pnT[o + bi * 32:o + bi * 32 + 32, bj * 32:bj * 32 + 32],
                    pnn[o + bj * 32:o + bj * 32 + 32, bi * 32:bi * 32 + 32])


def _build():
    nc = bacc.Bacc("TRN2", target_bir_lowering=False, debug=False,
                   num_devices=8)

    xT_d = nc.dram_tensor("xT", [D, HW], F32R, kind="ExternalInput").ap()
    wq_d = nc.dram_tensor("wqT", [D, D], F32R, kind="ExternalInput").ap()
    wk_d = nc.dram_tensor("wkT", [D, D], F32R, kind="ExternalInput").ap()
    wv_d = nc.dram_tensor("wvT", [D, D], F32R, kind="ExternalInput").ap()
    wo_d = nc.dram_tensor("woT", [D, D], BF16, kind="ExternalInput").ap()
    bq_d = nc.dram_tensor("bqt", [P, EO], F32, kind="ExternalInput").ap()
    bk_d = nc.dram_tensor("bkt", [P, EO], F32, kind="ExternalInput").ap()
    bo_d = nc.dram_tensor("bot", [P, EO], F32, kind="ExternalInput").ap()
    bm_d = nc.dram_tensor("bm", [P, 64], F32, kind="ExternalInput").ap()
    out_d = nc.dram_tensor("outT", [D, HW], BF16, kind="ExternalOutput").ap()

    xTv = xT_d.rearrange("(do p) t -> p do t", p=P)      # [128, 8, 4096]
    wqv = wq_d.rearrange("(do p) e -> p do e", p=P)
    wkv = wk_d.rearrange("(do p) e -> p do e", p=P)
    wvv = wv_d.rearrange("(do p) e -> p do e", p=P)
    wov = wo_d.rearrange("(do p) e -> p do e", p=P)
    outv = out_d.rearrange("(eo p) t -> p eo t", p=P)

    with tile.TileContext(nc) as tc:
      with tc.tile_pool(name="dram", bufs=1, space="DRAM") as dpool, \
           tc.tile_pool(name="consts", bufs=1) as cpool:
        # block-tiled token layout [p, w_blk, h, w_in, eo]: eo innermost
        # makes pass-A writes 2KB-contiguous and pass-B reads 16KB-contiguous
        qn_d = dpool.tile([P, 8, H, 8, EO], F32R)
        kn_d = dpool.tile([P, 8, H, 8, EO], F32R)
        vn_d = dpool.tile([HW, D], BF16)          # V natural row-order
        rc_d = dpool.tile([P, 8, H, 8, EO], BF16)

        bm_sb = cpool.tile([P, 64], F32)
        nc.sync.dma_start(bm_sb[:], bm_d)
        bo_sb = cpool.tile([P, EO], F32)
        nc.sync.dma_start(bo_sb[:], bo_d)

        # ---------------- pass A: projections + row attention ----------------
        with tc.tile_pool(name="wA", bufs=1) as wA, \
             tc.tile_pool(name="pA2", bufs=2) as pA2, \
             tc.tile_pool(name="pPm", bufs=1) as pPm, \
             tc.tile_pool(name="pA1", bufs=2) as pA1, \
             tc.tile_pool(name="pQK", bufs=1) as pQK, \
             tc.tile_pool(name="pSm", bufs=3) as pSm, \
             tc.tile_pool(name="psProj", bufs=2, space="PSUM") as psProj, \
             tc.tile_pool(name="psSc", bufs=2, space="PSUM") as psSc, \
             tc.tile_pool(name="psAv", bufs=2, space="PSUM") as psAv:
            wq_sb = wA.tile([P, EO, D], F32R)
            wk_sb = wA.tile([P, EO, D], F32R)
            wv_sb = wA.tile([P, EO, D], F32R)
            bq_sb = wA.tile([P, EO], F32)
            bk_sb = wA.tile([P, EO], F32)
            for esl_i in range(EO):
                nc.sync.dma_start(wq_sb[:, :, esl_i * P:(esl_i + 1) * P],
                                  wqv[:, :, esl_i * P:(esl_i + 1) * P])
            nc.sync.dma_start(bq_sb[:], bq_d)
            nc.sync.dma_start(bk_sb[:], bk_d)

            for c in range(NCH):
                tsl = slice(c * CH, (c + 1) * CH)
                x_sb = pA2.tile([P, EO, CH], F32R, tag="x")
                nc.sync.dma_start(x_sb[:], xTv[:, :, tsl])
                if c == 0:
                    # stagger the big weight loads behind chunk 0's x so the
                    # first Q-projection starts ~30us earlier
                    nc.sync.dma_start(wk_sb[:], wkv)
                    nc.sync.dma_start(wv_sb[:], wvv)

                q_sb = pQK.tile([P, EO, CH], F32R, tag="q")
                k_sb = pQK.tile([P, EO, CH], F32R, tag="k")
                for et in range(EO):
                    esl = slice(et * P, (et + 1) * P)
                    psq = psProj.tile([P, CH], F32, tag="pp")
                    for dt_ in range(EO):
                        nc.tensor.matmul(psq[:], wq_sb[:, dt_, esl],
                                         x_sb[:, dt_, :],
                                         start=(dt_ == 0), stop=(dt_ == EO - 1))
                    nc.scalar.add(q_sb[:, et, :], psq[:], add=bq_sb[:, et:et + 1])
                    psk = psProj.tile([P, CH], F32, tag="pp")
                    for dt_ in range(EO):
                        nc.tensor.matmul(psk[:], wk_sb[:, dt_, esl],
                                         x_sb[:, dt_, :],
                                         start=(dt_ == 0), stop=(dt_ == EO - 1))
                    nc.scalar.add(k_sb[:, et, :], psk[:], add=bk_sb[:, et:et + 1])

                # V natural [t, e] bf16
                v_sb = pQK.tile([P, CH // P, D], BF16, tag="v")
                for tt in range(CH // P):
                    for eh in range(2):
                        psv = psProj.tile([P, 512], F32, tag="pp")
                        for dt_ in range(EO):
                            nc.tensor.matmul(
                                psv[:], x_sb[:, dt_, tt * P:(tt + 1) * P],
                                wv_sb[:, dt_, eh * 512:(eh + 1) * 512],
                                start=(dt_ == 0), stop=(dt_ == EO - 1))
                        nc.scalar.copy(v_sb[:, tt, eh * 512:(eh + 1) * 512], psv[:])
                nc.sync.dma_start(
                    vn_d[tsl, :].rearrange("(tt p) e -> p tt e", p=P), v_sb[:])

                # permute q/k to (wb, h, wi, eo) order for pass-B reads
                hsl = slice(c * 8, (c + 1) * 8)
                for src_sb, dst_d in ((q_sb, qn_d), (k_sb, kn_d)):
                    qp_sb = pPm.tile([P, 8, 8, 8, EO], F32R, tag="qp")
                    for wb in range(8):
                        nc.vector.tensor_copy(
                            qp_sb[:, wb],
                            src_sb[:].rearrange(
                                "p eo (h wb wi) -> p wb h wi eo",
                                h=8, wb=8)[:, wb])
                    nc.sync.dma_start(dst_d[:, :, hsl, :, :], qp_sb[:])

                # row attention for the 8 h's of this chunk, in pairs
                # rout layout [p, wb, h8(chunk-local), wi, ds]
                rout_sb = pQK.tile([P, 8, 8, 8, EO], BF16, tag="rout")
                for pr in range(4):
                    psl = slice(pr * P, (pr + 1) * P)
                    pss = psSc.tile([P, P], F32, tag="sc")
                    for et in range(EO):
                        nc.tensor.matmul(pss[:], q_sb[:, et, psl],
                                         k_sb[:, et, psl],
                                         start=(et == 0), stop=(et == EO - 1))
                    pnT = pSm.tile([P, 64], BF16, tag="sm_pnT")
                    _softmax_block(nc, pSm,
                                   [pss[0:64, 0:64], pss[64:128, 64:128]],
                                   bm_sb, pnT)
                    psav = psAv.tile([P, 1024], F32, tag="av")
                    for hl in range(2):
                        h_loc = pr * 2 + hl
                        vp = (h_loc % 2) * 64
                        tt = h_loc // 2
                        for ds_ in range(EO):
                            nc.tensor.matmul(
                                psav[:, ds_ * P + hl * 64:ds_ * P + hl * 64 + 64],
                                v_sb[vp:vp + 64, tt, ds_ * P:(ds_ + 1) * P],
                                pnT[hl * 64:hl * 64 + 64, :],
                                start=True, stop=True, skip_group_check=True)
                    for hl in range(2):
                        nc.scalar.copy(
                            rout_sb[:, :, pr * 2 + hl, :, :],
                            psav[:].rearrange(
                                "p (ds hl wb wi) -> p hl wb wi ds",
                                ds=8, hl=2, wb=8)[:, hl])
                nc.sync.dma_start(rc_d[:, :, hsl, :, :], rout_sb[:])

        # ---------------- pass B: col attention + output projection ----------
        with tc.tile_pool(name="wB", bufs=1) as wB, \
             tc.tile_pool(name="pB2", bufs=2) as pB2, \
             tc.tile_pool(name="pBg", bufs=2) as pBg, \
             tc.tile_pool(name="pB1", bufs=2) as pB1, \
             tc.tile_pool(name="pB1s", bufs=1) as pB1s, \
             tc.tile_pool(name="pSmB", bufs=3) as pSm, \
             tc.tile_pool(name="psProjB", bufs=2, space="PSUM") as psProj, \
             tc.tile_pool(name="psScB", bufs=2, space="PSUM") as psSc, \
             tc.tile_pool(name="psAvB", bufs=2, space="PSUM") as psAv:
            wo_sb = wB.tile([P, EO, D], BF16)
            for et in range(EO):
                nc.sync.dma_start(wo_sb[:, :, et * P:(et + 1) * P],
                                  wov[:, :, et * P:(et + 1) * P])

            vn_v = vn_d[:].rearrange("(h w) e -> h w e", w=64)
            for c in range(NCH):
                tsl = slice(c * CH, (c + 1) * CH)
                wsl = slice(c * 8, (c + 1) * 8)
                # chunk layout: free = (eo, h, wl) -- 8 image columns
                qg_sb = pBg.tile([P, 64, 8, EO], F32R, tag="qg")
                kc_sb = pBg.tile([P, 64, 8, EO], F32R, tag="kc")
                nc.sync.dma_start(qg_sb[:], qn_d[:, c, :, :, :])
                nc.sync.dma_start(kc_sb[:], kn_d[:, c, :, :, :])
                # permute Q to (eo, wl, h) so score lhsT pair slices are
                # single contiguous free dims
                qc_sb = pB2.tile([P, EO, CH], F32R, tag="qc")
                nc.vector.tensor_copy(
                    qc_sb[:].rearrange("p eo (w h) -> p eo w h", w=8),
                    qg_sb[:].rearrange("p h w eo -> p eo w h"))
                kcp_sb = pB2.tile([P, EO, CH], F32R, tag="kcp")
                nc.scalar.copy(
                    kcp_sb[:].rearrange("p eo (w h) -> p eo w h", w=8),
                    kc_sb[:].rearrange("p h w eo -> p eo w h"))
                vcw = pB2.tile([P, 4, D], BF16, tag="vcw")
                for wl in range(8):
                    w_abs = c * 8 + wl
                    nc.sync.dma_start(
                        vcw[(wl % 2) * 64:(wl % 2) * 64 + 64, wl // 2, :],
                        vn_v[:, w_abs, :])
                rc_sb = pB1.tile([P, 64, 8, EO], BF16, tag="rc")
                nc.sync.dma_start(rc_sb[:], rc_d[:, c, :, :, :])

                sum_sb = pB1s.tile([P, EO, CH], BF16, tag="sum")
                for pr in range(4):
                    psl = slice(pr * P, (pr + 1) * P)
                    wpr = slice(pr * 2, pr * 2 + 2)
                    pss = psSc.tile([P, P], F32, tag="sc")
                    for et in range(EO):
                        nc.tensor.matmul(pss[:], qc_sb[:, et, psl],
                                         kcp_sb[:, et, psl],
                                         start=(et == 0), stop=(et == EO - 1))
                    pnT = pSm.tile([P, 64], BF16, tag="sm_pnT")
                    _softmax_block(nc, pSm,
                                   [pss[0:64, 0:64], pss[64:128, 64:128]],
                                   bm_sb, pnT)
                    psav = psAv.tile([P, 1024], F32, tag="av")
                    for wl2 in range(2):
                        w_loc = pr * 2 + wl2
                        vp = (w_loc % 2) * 64
                        wo_ = w_loc // 2
                        for ds_ in range(EO):
                            nc.tensor.matmul(
                                psav[:, ds_ * P + wl2 * 64:ds_ * P + wl2 * 64 + 64],
                                vcw[vp:vp + 64, wo_, ds_ * P:(ds_ + 1) * P],
                                pnT[wl2 * 64:wl2 * 64 + 64, :],
                                start=True, stop=True, skip_group_check=True)
                    nc.vector.tensor_tensor(
                        sum_sb[:, :, psl].rearrange("p ds (wl h) -> p ds wl h",
                                                    wl=2),
                        psav[:].rearrange("p (ds wl h) -> p ds wl h",
                                          ds=8, wl=2),
                        rc_sb[:, :, wpr, :].rearrange("p h w ds -> p ds w h"),
                        ALU.add)

                outT_sb = pB1s.tile([P, EO, CH], BF16, tag="out")
                for et in range(EO):
                    esl = slice(et * P, (et + 1) * P)
                    pso = psProj.tile([P, CH], F32, tag="po")
                    for dt_ in range(EO):
                        nc.tensor.matmul(pso[:], wo_sb[:, dt_, esl],
                                         sum_sb[:, dt_, :],
                                         start=(dt_ == 0), stop=(dt_ == EO - 1))
                    nc.scalar.add(outT_sb[:, et, :], pso[:],
                                  add=bo_sb[:, et:et + 1])
                nc.sync.dma_start(outv[:, :, tsl], outT_sb[:])

    nc.compile()
    return nc


def _prep_maps(x, Wq, bq, Wk, bk, Wv, bv, Wo, bo, sigma):
    gw = 1.0 / (2.0 * float(sigma[0]) ** 2)
    i = np.arange(64, dtype=np.float32)
    dist = np.square(i[:, None] - i[None, :])
    bm_half = (-gw * dist).astype(np.float32)          # [64, 64]
    bm = np.concatenate([bm_half, bm_half], axis=0)    # [128, 64] both halves

    bf = ml_dtypes.bfloat16
    wqT = np.ascontiguousarray(Wq.T)
    wkT = np.ascontiguousarray(Wk.T)
    wvT = np.ascontiguousarray(Wv.T)
    woT = np.ascontiguousarray(Wo.T).astype(bf)
    # fold bv: softmax rows sum to 1 -> out += 2 * bv @ Wo^T
    bo_eff = bo + 2.0 * (Wo @ bv)
    bqt = np.ascontiguousarray(bq.reshape(EO, P).T)
    bkt = np.ascontiguousarray(bk.reshape(EO, P).T)
    bot = np.ascontiguousarray(bo_eff.astype(np.float32).reshape(EO, P).T)

    in_maps = []
    for b in range(B):
        xT = np.ascontiguousarray(x[b].reshape(HW, D).T)
        in_maps.append({
            "xT": xT, "wqT": wqT, "wkT": wkT, "wvT": wvT, "woT": woT,
            "bqt": bqt, "bkt": bkt, "bot": bot, "bm": bm,
        })
    return in_maps


def kernel(x, Wq, bq, Wk, bk, Wv, bv, Wo, bo, sigma, **_ignored):
    x = np.asarray(x, np.float32)
    Wq = np.asarray(Wq, np.float32)
    Wk = np.asarray(Wk, np.float32)
    Wv = np.asarray(Wv, np.float32)
    Wo = np.asarray(Wo, np.float32)
    bq = np.asarray(bq, np.float32)
    bk = np.asarray(bk, np.float32)
    bv = np.asarray(bv, np.float32)
    bo = np.asarray(bo, np.float32)
    sigma = np.asarray(sigma, np.float32)

    if "nc" not in _cache:
        _cache["nc"] = _build()
    nc = _cache["nc"]

    in_maps = _prep_maps(x, Wq, bq, Wk, bk, Wv, bv, Wo, bo, sigma)

    trace = bool(int(os.environ.get("GSA_TRACE", "0")))
    ncore = int(os.environ.get("GSA_CORES", str(B)))
    res = bass_utils.run_bass_kernel_spmd(
        nc, in_maps[:ncore], core_ids=list(range(ncore)),
        trace=trace, trace_cores=[0] if trace else None)
    _cache["last_results"] = res

    out = np.zeros((B, H, W, D), dtype=np.float32)
    for b in range(ncore):
        oT = np.asarray(res.results[b]["outT"], dtype=np.float32)
        out[b] = oT.reshape(D, W, H).transpose(2, 1, 0)
    return out


# revision 3
# speedup vs baseline: 1.0231x; 1.0231x over previous
"""GSA layer (Gaussian-biased axial attention) Trainium2 Bass kernel, v2.

Full inputs in, full output out. Shards batch B=8 across 8 NeuronCores
(data parallel, one image per core). Self-contained: hardcodes shapes.

v2: all-bf16 matmul pipeline (fp32 matmuls are 4 cycles/row on TRN2,
bf16 is 1). PSUM accumulation and softmax stay fp32. bf16 intermediates
in DRAM halve pass-B DMA traffic.

Per-core dataflow (image = 64x64 tokens, D=1024):
  pass A (row):  stream xT chunks (512 tokens = 8 image rows):
                 Q,K proj ([e,t] layout), V proj ([t,e] layout),
                 fused row attention (scores bf16 paired matmuls, softmax
                 fp32 on free axis, DVE block transposes, bf16 AV),
                 writes to DRAM: V natural, row-major Q/K, r_outT.
  pass B (col):  per 8-column chunk: col attention (mirrored),
                 add r_outT, fused output projection -> outT col-order.
Host: transposes/reshapes, folds bv into output bias, unshards output.
"""

import os
import numpy as np
import ml_dtypes

import concourse.bass as bass
import concourse.mybir as mybir
import concourse.tile as tile
from concourse import bacc
from concourse import bass_utils

F32 = mybir.dt.float32
F32R = mybir.dt.float32r
BF16 = mybir.dt.bfloat16
AX = mybir.AxisListType
ALU = mybir.AluOpType
ACTF = mybir.ActivationFunctionType

B, H, W, D = 8, 64, 64, 1024
P = 128
HW = H * W            # 4096 tokens per image
CH = 512              # token chunk (8 image rows / 8 image cols)
NCH = HW // CH        # 8 chunks
EO = D // P           # 8 partition tiles of the 1024 dim

_cache = {}


def _softmax_block(nc, pool, half_aps, bm_sb, pnT):
    """Softmax over the free axis of two [64,64] PSUM score blocks
    (half_aps[hl], partition base hl*64), then write transposed bf16
    weights into pnT [128, 64] (half hl at rows hl*64:hl*64+64)."""
    sc = pool.tile([P, 64], F32, tag="sm_sc")
    negm = pool.tile([P, 1], F32, tag="sm_negm")
    ssum = pool.tile([P, 1], F32, tag="sm_ssum")
    rinv = pool.tile([P, 1], F32, tag="sm_rinv")
    pn = pool.tile([P, 64], F32, tag="sm_pn")
    pnn = pool.tile([P, 64], BF16, tag="sm_pnn")
    for hl in range(2):
        blk = slice(hl * 64, hl * 64 + 64)
        nc.vector.tensor_tensor(sc[blk, :], half_aps[hl],
                                bm_sb[blk, :], ALU.add)
        nc.vector.tensor_reduce(negm[blk, :], sc[blk, :], axis=AX.X,
                                op=ALU.max, negate=True)
        nc.scalar.activation(pn[blk, :], sc[blk, :], ACTF.Exp,
                             bias=negm[blk, 0:1], accum_out=ssum[blk, 0:1])
    nc.vector.reciprocal(rinv[:], ssum[:])
    nc.vector.tensor_scalar_mul(pnn[:], pn[:], rinv[:, 0:1])
    # transpose each 64x64 half via 4 DVE 32x32 block transposes
    for hl in range(2):
        o = hl * 64
        for bi in range(2):
            for bj in range(2):
                nc.vector.transpose(
                    pnT[o + bi * 32:o + bi * 32 + 32, bj * 32:bj * 32 + 32],
                    pnn[o + bj * 32:o + bj * 32 + 32, bi * 32:bi * 32 + 32])


def _build():
    nc = bacc.Bacc("TRN2", target_bir_lowering=False, debug=False,
                   num_devices=8)

    xT_d = nc.dram_tensor("xT", [D, HW], F32R, kind="ExternalInput").ap()
    wq_d = nc.dram_tensor("wqT", [D, D], F32R, kind="ExternalInput").ap()
    wk_d = nc.dram_tensor("wkT", [D, D], F32R, kind="ExternalInput").ap()
    wv_d = nc.dram_tensor("wvT", [D, D], F32R, kind="ExternalInput").ap()
    wo_d = nc.dram_tensor("woT", [D, D], BF16, kind="ExternalInput").ap()
    bq_d = nc.dram_tensor("bqt", [P, EO], F32, kind="ExternalInput").ap()
    bk_d = nc.dram_tensor("bkt", [P, EO], F32, kind="ExternalInput").ap()
    bo_d = nc.dram_tensor("bot", [P, EO], F32, kind="ExternalInput").ap()
    bm_d = nc.dram_tensor("bm", [P, 64], F32, kind="ExternalInput").ap()
    out_d = nc.dram_tensor("outT", [D, HW], BF16, kind="ExternalOutput").ap()

    xTv = xT_d.rearrange("(do p) t -> p do t", p=P)      # [128, 8, 4096]
    wqv = wq_d.rearrange("(do p) e -> p do e", p=P)
    wkv = wk_d.rearrange("(do p) e -> p do e", p=P)
    wvv = wv_d.rearrange("(do p) e -> p do e", p=P)
    wov = wo_d.rearrange("(do p) e -> p do e", p=P)
    outv = out_d.rearrange("(eo p) t -> p eo t", p=P)

    with tile.TileContext(nc) as tc:
      with tc.tile_pool(name="dram", bufs=1, space="DRAM") as dpool, \
           tc.tile_pool(name="consts", bufs=1) as cpool:
        # block-tiled token layout [p, w_blk, h, w_in, eo]: eo innermost
        # makes pass-A writes 2KB-contiguous and pass-B reads 16KB-contiguous
        qn_d = dpool.tile([P, 8, H, 8, EO], F32R)
        kn_d = dpool.tile([P, 8, H, 8, EO], F32R)
        vn_d = dpool.tile([HW, D], BF16)          # V natural row-order
        rc_d = dpool.tile([P, 8, H, 8, EO], BF16)

        bm_sb = cpool.tile([P, 64], F32)
        nc.sync.dma_start(bm_sb[:], bm_d)
        bo_sb = cpool.tile([P, EO], F32)
        nc.sync.dma_start(bo_sb[:], bo_d)

        # ---------------- pass A: projections + row attention ----------------
        with tc.tile_pool(name="wA", bufs=1) as wA, \
             tc.tile_pool(name="pA2", bufs=2) as pA2, \
             tc.tile_pool(name="pPm", bufs=1) as pPm, \
             tc.tile_pool(name="pA1", bufs=2) as pA1, \
             tc.tile_pool(name="pQK", bufs=1) as pQK, \
             tc.tile_pool(name="pSm", bufs=3) as pSm, \
             tc.tile_pool(name="psProj", bufs=2, space="PSUM") as psProj, \
             tc.tile_pool(name="psSc", bufs=2, space="PSUM") as psSc, \
             tc.tile_pool(name="psAv", bufs=2, space="PSUM") as psAv:
            wq_sb = wA.tile([P, EO, D], F32R)
            wk_sb = wA.tile([P, EO, D], F32R)
            wv_sb = wA.tile([P, EO, D], F32R)
            bq_sb = wA.tile([P, EO], F32)
            bk_sb = wA.tile([P, EO], F32)
            x0_sb = pA2.tile([P, EO, CH], F32R, tag="x")
            nc.sync.dma_start(x0_sb[:], xTv[:, :, 0:CH])
            nc.sync.dma_start(bq_sb[:], bq_d)
            nc.sync.dma_start(bk_sb[:], bk_d)
            for esl_i in range(EO):
                esl2 = slice(esl_i * P, (esl_i + 1) * P)
                nc.sync.dma_start(wq_sb[:, :, esl2], wqv[:, :, esl2])
                nc.sync.dma_start(wk_sb[:, :, esl2], wkv[:, :, esl2])

            for c in range(NCH):
                tsl = slice(c * CH, (c + 1) * CH)
                if c == 0:
                    x_sb = x0_sb
                    nc.sync.dma_start(wv_sb[:], wvv)
                else:
                    x_sb = pA2.tile([P, EO, CH], F32R, tag="x")
                    nc.sync.dma_start(x_sb[:], xTv[:, :, tsl])

                q_sb = pQK.tile([P, EO, CH], F32R, tag="q")
                k_sb = pQK.tile([P, EO, CH], F32R, tag="k")
                for et in range(EO):
                    esl = slice(et * P, (et + 1) * P)
                    psq = psProj.tile([P, CH], F32, tag="pp")
                    for dt_ in range(EO):
                        nc.tensor.matmul(psq[:], wq_sb[:, dt_, esl],
                                         x_sb[:, dt_, :],
                                         start=(dt_ == 0), stop=(dt_ == EO - 1))
                    nc.scalar.add(q_sb[:, et, :], psq[:], add=bq_sb[:, et:et + 1])
                    psk = psProj.tile([P, CH], F32, tag="pp")
                    for dt_ in range(EO):
                        nc.tensor.matmul(psk[:], wk_sb[:, dt_, esl],
                                         x_sb[:, dt_, :],
                                         start=(dt_ == 0), stop=(dt_ == EO - 1))
                    nc.scalar.add(k_sb[:, et, :], psk[:], add=bk_sb[:, et:et + 1])

                # V natural [t, e] bf16
                v_sb = pQK.tile([P, CH // P, D], BF16, tag="v")
                for tt in range(CH // P):
                    for eh in range(2):
                        psv = psProj.tile([P, 512], F32, tag="pp")
                        for dt_ in range(EO):
                            nc.tensor.matmul(
                                psv[:], x_sb[:, dt_, tt * P:(tt + 1) * P],
                                wv_sb[:, dt_, eh * 512:(eh + 1) * 512],
                                start=(dt_ == 0), stop=(dt_ == EO - 1))
                        nc.scalar.copy(v_sb[:, tt, eh * 512:(eh + 1) * 512], psv[:])
                nc.sync.dma_start(
                    vn_d[tsl, :].rearrange("(tt p) e -> p tt e", p=P), v_sb[:])

                # permute q/k to (wb, h, wi, eo) order for pass-B reads
                hsl = slice(c * 8, (c + 1) * 8)
                for src_sb, dst_d in ((q_sb, qn_d), (k_sb, kn_d)):
                    qp_sb = pPm.tile([P, 8, 8, 8, EO], F32R, tag="qp")
                    for wb in range(8):
                        nc.vector.tensor_copy(
                            qp_sb[:, wb],
                            src_sb[:].rearrange(
                                "p eo (h wb wi) -> p wb h wi eo",
                                h=8, wb=8)[:, wb])
                    nc.sync.dma_start(dst_d[:, :, hsl, :, :], qp_sb[:])

                # row attention for the 8 h's of this chunk, in pairs
                # rout layout [p, wb, h8(chunk-local), wi, ds]
                rout_sb = pQK.tile([P, 8, 8, 8, EO], BF16, tag="rout")
                for pr in range(4):
                    psl = slice(pr * P, (pr + 1) * P)
                    pss = psSc.tile([P, P], F32, tag="sc")
                    for et in range(EO):
                        nc.tensor.matmul(pss[:], q_sb[:, et, psl],
                                         k_sb[:, et, psl],
                                         start=(et == 0), stop=(et == EO - 1))
                    pnT = pSm.tile([P, 64], BF16, tag="sm_pnT")
                    _softmax_block(nc, pSm,
                                   [pss[0:64, 0:64], pss[64:128, 64:128]],
                                   bm_sb, pnT)
                    psav = psAv.tile([P, 1024], F32, tag="av")
                    for hl in range(2):
                        h_loc = pr * 2 + hl
                        vp = (h_loc % 2) * 64
                        tt = h_loc // 2
                        for ds_ in range(EO):
                            nc.tensor.matmul(
                                psav[:, ds_ * P + hl * 64:ds_ * P + hl * 64 + 64],
                                v_sb[vp:vp + 64, tt, ds_ * P:(ds_ + 1) * P],
                                pnT[hl * 64:hl * 64 + 64, :],
                                start=True, stop=True, skip_group_check=True)
                    for hl in range(2):
                        nc.scalar.copy(
                            rout_sb[:, :, pr * 2 + hl, :, :],
                            psav[:].rearrange(
                                "p (ds hl wb wi) -> p hl wb wi ds",
                                ds=8, hl=2, wb=8)[:, hl])
                nc.sync.dma_start(rc_d[:, :, hsl, :, :], rout_sb[:])

        # ---------------- pass B: col attention + output projection ----------
        with tc.tile_pool(name="wB", bufs=1) as wB, \
             tc.tile_pool(name="pB2", bufs=2) as pB2, \
             tc.tile_pool(name="pBg", bufs=1) as pBg, \
             tc.tile_pool(name="pB1", bufs=2) as pB1, \
             tc.tile_pool(name="pB1s", bufs=2) as pB1s, \
             tc.tile_pool(name="pBq", bufs=2) as pBq, \
             tc.tile_pool(name="pSmB", bufs=3) as pSm, \
             tc.tile_pool(name="psProjB", bufs=2, space="PSUM") as psProj, \
             tc.tile_pool(name="psScB", bufs=2, space="PSUM") as psSc, \
             tc.tile_pool(name="psAvB", bufs=2, space="PSUM") as psAv:
            wo_sb = wB.tile([P, EO, D], BF16)

            vn_v = vn_d[:].rearrange("(h w) e -> h w e", w=64)
            for c in range(NCH):
                tsl = slice(c * CH, (c + 1) * CH)
                wsl = slice(c * 8, (c + 1) * 8)
                # chunk layout: free = (eo, h, wl) -- 8 image columns
                qg_sb = pBg.tile([P, 64, 8, EO], F32R, tag="qg")
                kc_sb = pBg.tile([P, 64, 8, EO], F32R, tag="kc")
                nc.sync.dma_start(qg_sb[:], qn_d[:, c, :, :, :])
                nc.sync.dma_start(kc_sb[:], kn_d[:, c, :, :, :])
                # permute Q to (eo, wl, h) so score lhsT pair slices are
                # single contiguous free dims
                qc_sb = pBq.tile([P, EO, CH], F32R, tag="qc")
                nc.vector.tensor_copy(
                    qc_sb[:].rearrange("p eo (w h) -> p eo w h", w=8),
                    qg_sb[:].rearrange("p h w eo -> p eo w h"))
                kcp_sb = pBq.tile([P, EO, CH], F32R, tag="kcp")
                nc.scalar.copy(
                    kcp_sb[:].rearrange("p eo (w h) -> p eo w h", w=8),
                    kc_sb[:].rearrange("p h w eo -> p eo w h"))
                vcw = pB2.tile([P, 4, D], BF16, tag="vcw")
                for wl in range(8):
                    w_abs = c * 8 + wl
                    nc.sync.dma_start(
                        vcw[(wl % 2) * 64:(wl % 2) * 64 + 64, wl // 2, :],
                        vn_v[:, w_abs, :])
                rc_sb = pB1.tile([P, 64, 8, EO], BF16, tag="rc")
                nc.sync.dma_start(rc_sb[:], rc_d[:, c, :, :, :])
                if c == 0:
                    # wo loads queued behind the first chunk's gathers: the
                    # first out-projection only needs slice 0 ~15us in
                    for et in range(EO):
                        nc.sync.dma_start(wo_sb[:, :, et * P:(et + 1) * P],
                                          wov[:, :, et * P:(et + 1) * P])

                sum_sb = pB1s.tile([P, EO, CH], BF16, tag="sum")
                for pr in range(4):
                    psl = slice(pr * P, (pr + 1) * P)
                    wpr = slice(pr * 2, pr * 2 + 2)
                    pss = psSc.tile([P, P], F32, tag="sc")
                    for et in range(EO):
                        nc.tensor.matmul(pss[:], qc_sb[:, et, psl],
                                         kcp_sb[:, et, psl],
                                         start=(et == 0), stop=(et == EO - 1))
                    pnT = pSm.tile([P, 64], BF16, tag="sm_pnT")
                    _softmax_block(nc, pSm,
                                   [pss[0:64, 0:64], pss[64:128, 64:128]],
                                   bm_sb, pnT)
                    psav = psAv.tile([P, 1024], F32, tag="av")
                    for wl2 in range(2):
                        w_loc = pr * 2 + wl2
                        vp = (w_loc % 2) * 64
                        wo_ = w_loc // 2
                        for ds_ in range(EO):
                            nc.tensor.matmul(
                                psav[:, ds_ * P + wl2 * 64:ds_ * P + wl2 * 64 + 64],
                                vcw[vp:vp + 64, wo_, ds_ * P:(ds_ + 1) * P],
                                pnT[wl2 * 64:wl2 * 64 + 64, :],
                                start=True, stop=True, skip_group_check=True)
                    nc.vector.tensor_tensor(
                        sum_sb[:, :, psl].rearrange("p ds (wl h) -> p ds wl h",
                                                    wl=2),
                        psav[:].rearrange("p (ds wl h) -> p ds wl h",
                                          ds=8, wl=2),
                        rc_sb[:, :, wpr, :].rearrange("p h w ds -> p ds w h"),
                        ALU.add)

                outT_sb = pB1s.tile([P, EO, CH], BF16, tag="out")
                for et in range(EO):
                    esl = slice(et * P, (et + 1) * P)
                    pso = psProj.tile([P, CH], F32, tag="po")
                    for dt_ in range(EO):
                        nc.tensor.matmul(pso[:], wo_sb[:, dt_, esl],
                                         sum_sb[:, dt_, :],
                                         start=(dt_ == 0), stop=(dt_ == EO - 1))
                    nc.scalar.add(outT_sb[:, et, :], pso[:],
                                  add=bo_sb[:, et:et + 1])
                nc.sync.dma_start(outv[:, :, tsl], outT_sb[:])

    nc.compile()
    return nc


def _prep_maps(x, Wq, bq, Wk, bk, Wv, bv, Wo, bo, sigma):
    gw = 1.0 / (2.0 * float(sigma[0]) ** 2)
    i = np.arange(64, dtype=np.float32)
    dist = np.square(i[:, None] - i[None, :])
    bm_half = (-gw * dist).astype(np.float32)          # [64, 64]
    bm = np.concatenate([bm_half, bm_half], axis=0)    # [128, 64] both halves

    bf = ml_dtypes.bfloat16
    wqT = np.ascontiguousarray(Wq.T)
    wkT = np.ascontiguousarray(Wk.T)
    wvT = np.ascontiguousarray(Wv.T)
    woT = np.ascontiguousarray(Wo.T).astype(bf)
    # fold bv: softmax rows sum to 1 -> out += 2 * bv @ Wo^T
    bo_eff = bo + 2.0 * (Wo @ bv)
    bqt = np.ascontiguousarray(bq.reshape(EO, P).T)
    bkt = np.ascontiguousarray(bk.reshape(EO, P).T)
    bot = np.ascontiguousarray(bo_eff.astype(np.float32).reshape(EO, P).T)

    in_maps = []
    for b in range(B):
        xT = np.ascontiguousarray(x[b].reshape(HW, D).T)
        in_maps.append({
            "xT": xT, "wqT": wqT, "wkT": wkT, "wvT": wvT, "woT": woT,
            "bqt": bqt, "bkt": bkt, "bot": bot, "bm": bm,
        })
    return in_maps


def kernel(x, Wq, bq, Wk, bk, Wv, bv, Wo, bo, sigma, **_ignored):
    x = np.asarray(x, np.float32)
    Wq = np.asarray(Wq, np.float32)
    Wk = np.asarray(Wk, np.float32)
    Wv = np.asarray(Wv, np.float32)
    Wo = np.asarray(Wo, np.float32)
    bq = np.asarray(bq, np.float32)
    bk = np.asarray(bk, np.float32)
    bv = np.asarray(bv, np.float32)
    bo = np.asarray(bo, np.float32)
    sigma = np.asarray(sigma, np.float32)

    if "nc" not in _cache:
        _cache["nc"] = _build()
    nc = _cache["nc"]

    in_maps = _prep_maps(x, Wq, bq, Wk, bk, Wv, bv, Wo, bo, sigma)

    trace = bool(int(os.environ.get("GSA_TRACE", "0")))
    ncore = int(os.environ.get("GSA_CORES", str(B)))
    res = bass_utils.run_bass_kernel_spmd(
        nc, in_maps[:ncore], core_ids=list(range(ncore)),
        trace=trace, trace_cores=[0] if trace else None)
    _cache["last_results"] = res

    out = np.zeros((B, H, W, D), dtype=np.float32)
    for b in range(ncore):
        oT = np.asarray(res.results[b]["outT"], dtype=np.float32)
        out[b] = oT.reshape(D, W, H).transpose(2, 1, 0)
    return out


# revision 4
# speedup vs baseline: 1.0495x; 1.0258x over previous
"""GSA layer (Gaussian-biased axial attention) Trainium2 Bass kernel, v2.

Full inputs in, full output out. Shards batch B=8 across 8 NeuronCores
(data parallel, one image per core). Self-contained: hardcodes shapes.

v2: all-bf16 matmul pipeline (fp32 matmuls are 4 cycles/row on TRN2,
bf16 is 1). PSUM accumulation and softmax stay fp32. bf16 intermediates
in DRAM halve pass-B DMA traffic.

Per-core dataflow (image = 64x64 tokens, D=1024):
  pass A (row):  stream xT chunks (512 tokens = 8 image rows):
                 Q,K proj ([e,t] layout), V proj ([t,e] layout),
                 fused row attention (scores bf16 paired matmuls, softmax
                 fp32 on free axis, DVE block transposes, bf16 AV),
                 writes to DRAM: V natural, row-major Q/K, r_outT.
  pass B (col):  per 8-column chunk: col attention (mirrored),
                 add r_outT, fused output projection -> outT col-order.
Host: transposes/reshapes, folds bv into output bias, unshards output.
"""

import os
import numpy as np
import ml_dtypes

import concourse.bass as bass
import concourse.mybir as mybir
import concourse.tile as tile
from concourse import bacc
from concourse import bass_utils

F32 = mybir.dt.float32
F32R = mybir.dt.float32r
BF16 = mybir.dt.bfloat16
AX = mybir.AxisListType
ALU = mybir.AluOpType
ACTF = mybir.ActivationFunctionType

B, H, W, D = 8, 64, 64, 1024
P = 128
HW = H * W            # 4096 tokens per image
CH = 512              # token chunk (8 image rows / 8 image cols)
NCH = HW // CH        # 8 chunks
EO = D // P           # 8 partition tiles of the 1024 dim

_cache = {}


def _softmax_block(nc, pool, half_aps, bm_sb, pnT):
    """Softmax over the free axis of two [64,64] PSUM score blocks
    (half_aps[hl], partition base hl*64), then write transposed bf16
    weights into pnT [128, 64] (half hl at rows hl*64:hl*64+64)."""
    sc = pool.tile([P, 64], F32, tag="sm_sc")
    negm = pool.tile([P, 1], F32, tag="sm_negm")
    ssum = pool.tile([P, 1], F32, tag="sm_ssum")
    rinv = pool.tile([P, 1], F32, tag="sm_rinv")
    pn = pool.tile([P, 64], F32, tag="sm_pn")
    pnn = pool.tile([P, 64], BF16, tag="sm_pnn")
    for hl in range(2):
        blk = slice(hl * 64, hl * 64 + 64)
        nc.vector.tensor_tensor(sc[blk, :], half_aps[hl],
                                bm_sb[blk, :], ALU.add)
        nc.vector.tensor_reduce(negm[blk, :], sc[blk, :], axis=AX.X,
                                op=ALU.max, negate=True)
        nc.scalar.activation(pn[blk, :], sc[blk, :], ACTF.Exp,
                             bias=negm[blk, 0:1], accum_out=ssum[blk, 0:1])
    nc.vector.reciprocal(rinv[:], ssum[:])
    nc.vector.tensor_scalar_mul(pnn[:], pn[:], rinv[:, 0:1])
    # block-diagonal pnT [128, 128]: off-diag zeroed by caller; transpose
    # each 64x64 half into its diagonal block via 4 DVE 32x32 transposes
    for hl in range(2):
        o = hl * 64
        for bi in range(2):
            for bj in range(2):
                nc.vector.transpose(
                    pnT[o + bi * 32:o + bi * 32 + 32,
                        o + bj * 32:o + bj * 32 + 32],
                    pnn[o + bj * 32:o + bj * 32 + 32, bi * 32:bi * 32 + 32])


def _build():
    nc = bacc.Bacc("TRN2", target_bir_lowering=False, debug=False,
                   num_devices=8)

    xT_d = nc.dram_tensor("xT", [D, HW], F32R, kind="ExternalInput").ap()
    wq_d = nc.dram_tensor("wqT", [D, D], F32R, kind="ExternalInput").ap()
    wk_d = nc.dram_tensor("wkT", [D, D], F32R, kind="ExternalInput").ap()
    wv_d = nc.dram_tensor("wvT", [D, D], F32R, kind="ExternalInput").ap()
    wo_d = nc.dram_tensor("woT", [D, D], BF16, kind="ExternalInput").ap()
    bq_d = nc.dram_tensor("bqt", [P, EO], F32, kind="ExternalInput").ap()
    bk_d = nc.dram_tensor("bkt", [P, EO], F32, kind="ExternalInput").ap()
    bo_d = nc.dram_tensor("bot", [P, EO], F32, kind="ExternalInput").ap()
    bm_d = nc.dram_tensor("bm", [P, 64], F32, kind="ExternalInput").ap()
    out_d = nc.dram_tensor("outT", [D, HW], BF16, kind="ExternalOutput").ap()

    xTv = xT_d.rearrange("(do p) t -> p do t", p=P)      # [128, 8, 4096]
    wqv = wq_d.rearrange("(do p) e -> p do e", p=P)
    wkv = wk_d.rearrange("(do p) e -> p do e", p=P)
    wvv = wv_d.rearrange("(do p) e -> p do e", p=P)
    wov = wo_d.rearrange("(do p) e -> p do e", p=P)
    outv = out_d.rearrange("(eo p) t -> p eo t", p=P)

    with tile.TileContext(nc) as tc:
      with tc.tile_pool(name="dram", bufs=1, space="DRAM") as dpool, \
           tc.tile_pool(name="consts", bufs=1) as cpool:
        # block-tiled token layout [p, w_blk, h, w_in, eo]: eo innermost
        # makes pass-A writes 2KB-contiguous and pass-B reads 16KB-contiguous
        qn_d = dpool.tile([P, 8, H, 8, EO], F32R)
        kn_d = dpool.tile([P, 8, H, 8, EO], F32R)
        vn_d = dpool.tile([HW, D], BF16)          # V natural row-order
        rc_d = dpool.tile([P, 8, H, 8, EO], BF16)

        bm_sb = cpool.tile([P, 64], F32)
        nc.sync.dma_start(bm_sb[:], bm_d)
        bo_sb = cpool.tile([P, EO], F32)
        nc.sync.dma_start(bo_sb[:], bo_d)

        # ---------------- pass A: projections + row attention ----------------
        with tc.tile_pool(name="wA", bufs=1) as wA, \
             tc.tile_pool(name="pA2", bufs=2) as pA2, \
             tc.tile_pool(name="pPm", bufs=1) as pPm, \
             tc.tile_pool(name="pA1", bufs=2) as pA1, \
             tc.tile_pool(name="pQK", bufs=1) as pQK, \
             tc.tile_pool(name="pSm", bufs=3) as pSm, \
             tc.tile_pool(name="psProj", bufs=2, space="PSUM") as psProj, \
             tc.tile_pool(name="psSc", bufs=2, space="PSUM") as psSc, \
             tc.tile_pool(name="psAv", bufs=2, space="PSUM") as psAv:
            wq_sb = wA.tile([P, EO, D], F32R)
            wk_sb = wA.tile([P, EO, D], F32R)
            wv_sb = wA.tile([P, EO, D], F32R)
            bq_sb = wA.tile([P, EO], F32)
            bk_sb = wA.tile([P, EO], F32)
            x0_sb = pA2.tile([P, EO, CH], F32R, tag="x")
            nc.sync.dma_start(x0_sb[:], xTv[:, :, 0:CH])
            nc.sync.dma_start(bq_sb[:], bq_d)
            nc.sync.dma_start(bk_sb[:], bk_d)
            for esl_i in range(EO):
                esl2 = slice(esl_i * P, (esl_i + 1) * P)
                nc.sync.dma_start(wq_sb[:, :, esl2], wqv[:, :, esl2])
                nc.sync.dma_start(wk_sb[:, :, esl2], wkv[:, :, esl2])

            for c in range(NCH):
                tsl = slice(c * CH, (c + 1) * CH)
                if c == 0:
                    x_sb = x0_sb
                    nc.sync.dma_start(wv_sb[:], wvv)
                else:
                    x_sb = pA2.tile([P, EO, CH], F32R, tag="x")
                    nc.sync.dma_start(x_sb[:], xTv[:, :, tsl])

                q_sb = pQK.tile([P, EO, CH], F32R, tag="q")
                k_sb = pQK.tile([P, EO, CH], F32R, tag="k")
                for et in range(EO):
                    esl = slice(et * P, (et + 1) * P)
                    psq = psProj.tile([P, CH], F32, tag="pp")
                    for dt_ in range(EO):
                        nc.tensor.matmul(psq[:], wq_sb[:, dt_, esl],
                                         x_sb[:, dt_, :],
                                         start=(dt_ == 0), stop=(dt_ == EO - 1))
                    nc.scalar.add(q_sb[:, et, :], psq[:], add=bq_sb[:, et:et + 1])
                    psk = psProj.tile([P, CH], F32, tag="pp")
                    for dt_ in range(EO):
                        nc.tensor.matmul(psk[:], wk_sb[:, dt_, esl],
                                         x_sb[:, dt_, :],
                                         start=(dt_ == 0), stop=(dt_ == EO - 1))
                    nc.scalar.add(k_sb[:, et, :], psk[:], add=bk_sb[:, et:et + 1])

                # V natural [t, e] bf16
                v_sb = pQK.tile([P, CH // P, D], BF16, tag="v")
                for tt in range(CH // P):
                    for eh in range(2):
                        psv = psProj.tile([P, 512], F32, tag="pp")
                        for dt_ in range(EO):
                            nc.tensor.matmul(
                                psv[:], x_sb[:, dt_, tt * P:(tt + 1) * P],
                                wv_sb[:, dt_, eh * 512:(eh + 1) * 512],
                                start=(dt_ == 0), stop=(dt_ == EO - 1))
                        nc.scalar.copy(v_sb[:, tt, eh * 512:(eh + 1) * 512], psv[:])
                nc.sync.dma_start(
                    vn_d[tsl, :].rearrange("(tt p) e -> p tt e", p=P), v_sb[:])

                # permute q/k to (wb, h, wi, eo) order for pass-B reads
                hsl = slice(c * 8, (c + 1) * 8)
                for src_sb, dst_d in ((q_sb, qn_d), (k_sb, kn_d)):
                    qp_sb = pPm.tile([P, 8, 8, 8, EO], F32R, tag="qp")
                    for wb in range(8):
                        nc.vector.tensor_copy(
                            qp_sb[:, wb],
                            src_sb[:].rearrange(
                                "p eo (h wb wi) -> p wb h wi eo",
                                h=8, wb=8)[:, wb])
                    nc.sync.dma_start(dst_d[:, :, hsl, :, :], qp_sb[:])

                # row attention for the 8 h's of this chunk, in pairs
                # rout layout [p, wb, h8(chunk-local), wi, ds]
                rout_sb = pQK.tile([P, 8, 8, 8, EO], BF16, tag="rout")
                for pr in range(4):
                    psl = slice(pr * P, (pr + 1) * P)
                    pss = psSc.tile([P, P], F32, tag="sc")
                    for et in range(EO):
                        nc.tensor.matmul(pss[:], q_sb[:, et, psl],
                                         k_sb[:, et, psl],
                                         start=(et == 0), stop=(et == EO - 1))
                    pnT = pSm.tile([P, P], BF16, tag="sm_pnT")
                    nc.vector.memset(pnT[0:64, 64:128], 0)
                    nc.vector.memset(pnT[64:128, 0:64], 0)
                    _softmax_block(nc, pSm,
                                   [pss[0:64, 0:64], pss[64:128, 64:128]],
                                   bm_sb, pnT)
                    psav = psAv.tile([P, 1024], F32, tag="av")
                    for ds_ in range(EO):
                        nc.tensor.matmul(
                            psav[:, ds_ * P:(ds_ + 1) * P],
                            v_sb[:, pr, ds_ * P:(ds_ + 1) * P],
                            pnT[:],
                            start=True, stop=True, skip_group_check=True)
                    for hl in range(2):
                        nc.scalar.copy(
                            rout_sb[:, :, pr * 2 + hl, :, :],
                            psav[:].rearrange(
                                "p (ds hl wb wi) -> p hl wb wi ds",
                                ds=8, hl=2, wb=8)[:, hl])
                nc.sync.dma_start(rc_d[:, :, hsl, :, :], rout_sb[:])

        # ---------------- pass B: col attention + output projection ----------
        with tc.tile_pool(name="wB", bufs=1) as wB, \
             tc.tile_pool(name="pB2", bufs=2) as pB2, \
             tc.tile_pool(name="pBg", bufs=1) as pBg, \
             tc.tile_pool(name="pB1", bufs=2) as pB1, \
             tc.tile_pool(name="pB1s", bufs=2) as pB1s, \
             tc.tile_pool(name="pBq", bufs=2) as pBq, \
             tc.tile_pool(name="pSmB", bufs=3) as pSm, \
             tc.tile_pool(name="psProjB", bufs=2, space="PSUM") as psProj, \
             tc.tile_pool(name="psScB", bufs=2, space="PSUM") as psSc, \
             tc.tile_pool(name="psAvB", bufs=2, space="PSUM") as psAv:
            wo_sb = wB.tile([P, EO, D], BF16)

            vn_v = vn_d[:].rearrange("(h w) e -> h w e", w=64)
            for c in range(NCH):
                tsl = slice(c * CH, (c + 1) * CH)
                wsl = slice(c * 8, (c + 1) * 8)
                # chunk layout: free = (eo, h, wl) -- 8 image columns
                qg_sb = pBg.tile([P, 64, 8, EO], F32R, tag="qg")
                kc_sb = pBg.tile([P, 64, 8, EO], F32R, tag="kc")
                nc.sync.dma_start(qg_sb[:], qn_d[:, c, :, :, :])
                nc.sync.dma_start(kc_sb[:], kn_d[:, c, :, :, :])
                # permute Q to (eo, wl, h) so score lhsT pair slices are
                # single contiguous free dims
                qc_sb = pBq.tile([P, EO, CH], F32R, tag="qc")
                nc.vector.tensor_copy(
                    qc_sb[:].rearrange("p eo (w h) -> p eo w h", w=8),
                    qg_sb[:].rearrange("p h w eo -> p eo w h"))
                kcp_sb = pBq.tile([P, EO, CH], F32R, tag="kcp")
                nc.scalar.copy(
                    kcp_sb[:].rearrange("p eo (w h) -> p eo w h", w=8),
                    kc_sb[:].rearrange("p h w eo -> p eo w h"))
                vcw = pB2.tile([P, 4, D], BF16, tag="vcw")
                for wl in range(8):
                    w_abs = c * 8 + wl
                    nc.sync.dma_start(
                        vcw[(wl % 2) * 64:(wl % 2) * 64 + 64, wl // 2, :],
                        vn_v[:, w_abs, :])
                rc_sb = pB1.tile([P, 64, 8, EO], BF16, tag="rc")
                nc.sync.dma_start(rc_sb[:], rc_d[:, c, :, :, :])
                if c == 0:
                    # wo loads queued behind the first chunk's gathers: the
                    # first out-projection only needs slice 0 ~15us in
                    for et in range(EO):
                        nc.sync.dma_start(wo_sb[:, :, et * P:(et + 1) * P],
                                          wov[:, :, et * P:(et + 1) * P])

                sum_sb = pB1s.tile([P, EO, CH], BF16, tag="sum")
                for pr in range(4):
                    psl = slice(pr * P, (pr + 1) * P)
                    wpr = slice(pr * 2, pr * 2 + 2)
                    pss = psSc.tile([P, P], F32, tag="sc")
                    for et in range(EO):
                        nc.tensor.matmul(pss[:], qc_sb[:, et, psl],
                                         kcp_sb[:, et, psl],
                                         start=(et == 0), stop=(et == EO - 1))
                    pnT = pSm.tile([P, P], BF16, tag="sm_pnT")
                    nc.vector.memset(pnT[0:64, 64:128], 0)
                    nc.vector.memset(pnT[64:128, 0:64], 0)
                    _softmax_block(nc, pSm,
                                   [pss[0:64, 0:64], pss[64:128, 64:128]],
                                   bm_sb, pnT)
                    psav = psAv.tile([P, 1024], F32, tag="av")
                    for ds_ in range(EO):
                        nc.tensor.matmul(
                            psav[:, ds_ * P:(ds_ + 1) * P],
                            vcw[:, pr, ds_ * P:(ds_ + 1) * P],
                            pnT[:],
                            start=True, stop=True, skip_group_check=True)
                    nc.vector.tensor_tensor(
                        sum_sb[:, :, psl].rearrange("p ds (wl h) -> p ds wl h",
                                                    wl=2),
                        psav[:].rearrange("p (ds wl h) -> p ds wl h",
                                          ds=8, wl=2),
                        rc_sb[:, :, wpr, :].rearrange("p h w ds -> p ds w h"),
                        ALU.add)

                outT_sb = pB1s.tile([P, EO, CH], BF16, tag="out")
                for et in range(EO):
                    esl = slice(et * P, (et + 1) * P)
                    pso = psProj.tile([P, CH], F32, tag="po")
                    for dt_ in range(EO):
                        nc.tensor.matmul(pso[:], wo_sb[:, dt_, esl],
                                         sum_sb[:, dt_, :],
                                         start=(dt_ == 0), stop=(dt_ == EO - 1))
                    nc.scalar.add(outT_sb[:, et, :], pso[:],
                                  add=bo_sb[:, et:et + 1])
                nc.sync.dma_start(outv[:, :, tsl], outT_sb[:])

    nc.compile()
    return nc


def _prep_maps(x, Wq, bq, Wk, bk, Wv, bv, Wo, bo, sigma):
    gw = 1.0 / (2.0 * float(sigma[0]) ** 2)
    i = np.arange(64, dtype=np.float32)
    dist = np.square(i[:, None] - i[None, :])
    bm_half = (-gw * dist).astype(np.float32)          # [64, 64]
    bm = np.concatenate([bm_half, bm_half], axis=0)    # [128, 64] both halves

    bf = ml_dtypes.bfloat16
    wqT = np.ascontiguousarray(Wq.T)
    wkT = np.ascontiguousarray(Wk.T)
    wvT = np.ascontiguousarray(Wv.T)
    woT = np.ascontiguousarray(Wo.T).astype(bf)
    # fold bv: softmax rows sum to 1 -> out += 2 * bv @ Wo^T
    bo_eff = bo + 2.0 * (Wo @ bv)
    bqt = np.ascontiguousarray(bq.reshape(EO, P).T)
    bkt = np.ascontiguousarray(bk.reshape(EO, P).T)
    bot = np.ascontiguousarray(bo_eff.astype(np.float32).reshape(EO, P).T)

    in_maps = []
    for b in range(B):
        xT = np.ascontiguousarray(x[b].reshape(HW, D).T)
        in_maps.append({
            "xT": xT, "wqT": wqT, "wkT": wkT, "wvT": wvT, "woT": woT,
            "bqt": bqt, "bkt": bkt, "bot": bot, "bm": bm,
        })
    return in_maps


def kernel(x, Wq, bq, Wk, bk, Wv, bv, Wo, bo, sigma, **_ignored):
    x = np.asarray(x, np.float32)
    Wq = np.asarray(Wq, np.float32)
    Wk = np.asarray(Wk, np.float32)
    Wv = np.asarray(Wv, np.float32)
    Wo = np.asarray(Wo, np.float32)
    bq = np.asarray(bq, np.float32)
    bk = np.asarray(bk, np.float32)
    bv = np.asarray(bv, np.float32)
    bo = np.asarray(bo, np.float32)
    sigma = np.asarray(sigma, np.float32)

    if "nc" not in _cache:
        _cache["nc"] = _build()
    nc = _cache["nc"]

    in_maps = _prep_maps(x, Wq, bq, Wk, bk, Wv, bv, Wo, bo, sigma)

    trace = bool(int(os.environ.get("GSA_TRACE", "0")))
    ncore = int(os.environ.get("GSA_CORES", str(B)))
    res = bass_utils.run_bass_kernel_spmd(
        nc, in_maps[:ncore], core_ids=list(range(ncore)),
        trace=trace, trace_cores=[0] if trace else None)
    _cache["last_results"] = res

    out = np.zeros((B, H, W, D), dtype=np.float32)
    for b in range(ncore):
        oT = np.asarray(res.results[b]["outT"], dtype=np.float32)
        out[b] = oT.reshape(D, W, H).transpose(2, 1, 0)
    return out


# revision 5
# speedup vs baseline: 1.0553x; 1.0055x over previous
"""GSA layer (Gaussian-biased axial attention) Trainium2 Bass kernel, v2.

Full inputs in, full output out. Shards batch B=8 across 8 NeuronCores
(data parallel, one image per core). Self-contained: hardcodes shapes.

v2: all-bf16 matmul pipeline (fp32 matmuls are 4 cycles/row on TRN2,
bf16 is 1). PSUM accumulation and softmax stay fp32. bf16 intermediates
in DRAM halve pass-B DMA traffic.

Per-core dataflow (image = 64x64 tokens, D=1024):
  pass A (row):  stream xT chunks (512 tokens = 8 image rows):
                 Q,K proj ([e,t] layout), V proj ([t,e] layout),
                 fused row attention (scores bf16 paired matmuls, softmax
                 fp32 on free axis, DVE block transposes, bf16 AV),
                 writes to DRAM: V natural, row-major Q/K, r_outT.
  pass B (col):  per 8-column chunk: col attention (mirrored),
                 add r_outT, fused output projection -> outT col-order.
Host: transposes/reshapes, folds bv into output bias, unshards output.
"""

import os
import numpy as np
import ml_dtypes

import concourse.bass as bass
import concourse.mybir as mybir
import concourse.tile as tile
from concourse import bacc
from concourse import bass_utils

F32 = mybir.dt.float32
F32R = mybir.dt.float32r
F16 = mybir.dt.float16
BF16 = mybir.dt.bfloat16
AX = mybir.AxisListType
ALU = mybir.AluOpType
ACTF = mybir.ActivationFunctionType

B, H, W, D = 8, 64, 64, 1024
P = 128
HW = H * W            # 4096 tokens per image
CH = 512              # token chunk (8 image rows / 8 image cols)
NCH = HW // CH        # 8 chunks
EO = D // P           # 8 partition tiles of the 1024 dim

_cache = {}


def _softmax_block(nc, pool, half_aps, bm_sb, pnT):
    """Softmax over the free axis of two [64,64] PSUM score blocks
    (half_aps[hl], partition base hl*64), then write transposed bf16
    weights into pnT [128, 64] (half hl at rows hl*64:hl*64+64)."""
    sc = pool.tile([P, 64], F32, tag="sm_sc")
    negm = pool.tile([P, 1], F32, tag="sm_negm")
    ssum = pool.tile([P, 1], F32, tag="sm_ssum")
    rinv = pool.tile([P, 1], F32, tag="sm_rinv")
    pn = pool.tile([P, 64], F32, tag="sm_pn")
    pnn = pool.tile([P, 64], BF16, tag="sm_pnn")
    for hl in range(2):
        blk = slice(hl * 64, hl * 64 + 64)
        nc.vector.tensor_tensor(sc[blk, :], half_aps[hl],
                                bm_sb[blk, :], ALU.add)
        nc.vector.tensor_reduce(negm[blk, :], sc[blk, :], axis=AX.X,
                                op=ALU.max, negate=True)
        nc.scalar.activation(pn[blk, :], sc[blk, :], ACTF.Exp,
                             bias=negm[blk, 0:1], accum_out=ssum[blk, 0:1])
    nc.vector.reciprocal(rinv[:], ssum[:])
    nc.vector.tensor_scalar_mul(pnn[:], pn[:], rinv[:, 0:1])
    # block-diagonal pnT [128, 128]: off-diag zeroed by caller; transpose
    # each 64x64 half into its diagonal block via 4 DVE 32x32 transposes
    for hl in range(2):
        o = hl * 64
        for bi in range(2):
            for bj in range(2):
                nc.vector.transpose(
                    pnT[o + bi * 32:o + bi * 32 + 32,
                        o + bj * 32:o + bj * 32 + 32],
                    pnn[o + bj * 32:o + bj * 32 + 32, bi * 32:bi * 32 + 32])


def _build():
    nc = bacc.Bacc("TRN2", target_bir_lowering=False, debug=False,
                   num_devices=8)

    xT_d = nc.dram_tensor("xT", [D, HW], F16, kind="ExternalInput").ap()
    wq_d = nc.dram_tensor("wqT", [D, D], F16, kind="ExternalInput").ap()
    wk_d = nc.dram_tensor("wkT", [D, D], F16, kind="ExternalInput").ap()
    wv_d = nc.dram_tensor("wvT", [D, D], F16, kind="ExternalInput").ap()
    wo_d = nc.dram_tensor("woT", [D, D], BF16, kind="ExternalInput").ap()
    bq_d = nc.dram_tensor("bqt", [P, EO], F32, kind="ExternalInput").ap()
    bk_d = nc.dram_tensor("bkt", [P, EO], F32, kind="ExternalInput").ap()
    bo_d = nc.dram_tensor("bot", [P, EO], F32, kind="ExternalInput").ap()
    bm_d = nc.dram_tensor("bm", [P, 64], F32, kind="ExternalInput").ap()
    out_d = nc.dram_tensor("outT", [D, HW], BF16, kind="ExternalOutput").ap()

    xTv = xT_d.rearrange("(do p) t -> p do t", p=P)      # [128, 8, 4096]
    wqv = wq_d.rearrange("(do p) e -> p do e", p=P)
    wkv = wk_d.rearrange("(do p) e -> p do e", p=P)
    wvv = wv_d.rearrange("(do p) e -> p do e", p=P)
    wov = wo_d.rearrange("(do p) e -> p do e", p=P)
    outv = out_d.rearrange("(eo p) t -> p eo t", p=P)

    with tile.TileContext(nc) as tc:
      with tc.tile_pool(name="dram", bufs=1, space="DRAM") as dpool, \
           tc.tile_pool(name="consts", bufs=1) as cpool:
        # block-tiled token layout [p, w_blk, h, w_in, eo]: eo innermost
        # makes pass-A writes 2KB-contiguous and pass-B reads 16KB-contiguous
        qn_d = dpool.tile([P, 8, H, 8, EO], F32R)
        kn_d = dpool.tile([P, 8, H, 8, EO], F32R)
        vn_d = dpool.tile([HW, D], BF16)          # V natural row-order
        rc_d = dpool.tile([P, 8, H, 8, EO], BF16)

        bm_sb = cpool.tile([P, 64], F32)
        nc.sync.dma_start(bm_sb[:], bm_d)
        bo_sb = cpool.tile([P, EO], F32)
        nc.sync.dma_start(bo_sb[:], bo_d)

        # ---------------- pass A: projections + row attention ----------------
        with tc.tile_pool(name="wA", bufs=1) as wA, \
             tc.tile_pool(name="pA2", bufs=2) as pA2, \
             tc.tile_pool(name="pPm", bufs=1) as pPm, \
             tc.tile_pool(name="pA1", bufs=2) as pA1, \
             tc.tile_pool(name="pQK", bufs=1) as pQK, \
             tc.tile_pool(name="pSm", bufs=3) as pSm, \
             tc.tile_pool(name="psProj", bufs=2, space="PSUM") as psProj, \
             tc.tile_pool(name="psSc", bufs=2, space="PSUM") as psSc, \
             tc.tile_pool(name="psAv", bufs=2, space="PSUM") as psAv:
            wq_sb = wA.tile([P, EO, D], F16)
            wk_sb = wA.tile([P, EO, D], F16)
            wv_sb = wA.tile([P, EO, D], F16)
            bq_sb = wA.tile([P, EO], F32)
            bk_sb = wA.tile([P, EO], F32)
            x0_sb = pA2.tile([P, EO, CH], F16, tag="x")
            nc.sync.dma_start(x0_sb[:], xTv[:, :, 0:CH])
            nc.sync.dma_start(bq_sb[:], bq_d)
            nc.sync.dma_start(bk_sb[:], bk_d)
            for esl_i in range(EO):
                esl2 = slice(esl_i * P, (esl_i + 1) * P)
                nc.sync.dma_start(wq_sb[:, :, esl2], wqv[:, :, esl2])
                nc.sync.dma_start(wk_sb[:, :, esl2], wkv[:, :, esl2])

            for c in range(NCH):
                tsl = slice(c * CH, (c + 1) * CH)
                if c == 0:
                    x_sb = x0_sb
                    nc.sync.dma_start(wv_sb[:], wvv)
                else:
                    x_sb = pA2.tile([P, EO, CH], F16, tag="x")
                    nc.sync.dma_start(x_sb[:], xTv[:, :, tsl])

                q_sb = pQK.tile([P, EO, CH], F16, tag="q")
                k_sb = pQK.tile([P, EO, CH], F16, tag="k")
                for et in range(EO):
                    esl = slice(et * P, (et + 1) * P)
                    psq = psProj.tile([P, CH], F32, tag="pp")
                    for dt_ in range(EO):
                        nc.tensor.matmul(psq[:], wq_sb[:, dt_, esl],
                                         x_sb[:, dt_, :],
                                         start=(dt_ == 0), stop=(dt_ == EO - 1))
                    nc.scalar.add(q_sb[:, et, :], psq[:], add=bq_sb[:, et:et + 1])
                    psk = psProj.tile([P, CH], F32, tag="pp")
                    for dt_ in range(EO):
                        nc.tensor.matmul(psk[:], wk_sb[:, dt_, esl],
                                         x_sb[:, dt_, :],
                                         start=(dt_ == 0), stop=(dt_ == EO - 1))
                    nc.scalar.add(k_sb[:, et, :], psk[:], add=bk_sb[:, et:et + 1])

                # V natural [t, e] bf16
                v_sb = pQK.tile([P, CH // P, D], BF16, tag="v")
                for tt in range(CH // P):
                    for eh in range(2):
                        psv = psProj.tile([P, 512], F32, tag="pp")
                        for dt_ in range(EO):
                            nc.tensor.matmul(
                                psv[:], x_sb[:, dt_, tt * P:(tt + 1) * P],
                                wv_sb[:, dt_, eh * 512:(eh + 1) * 512],
                                start=(dt_ == 0), stop=(dt_ == EO - 1))
                        nc.scalar.copy(v_sb[:, tt, eh * 512:(eh + 1) * 512], psv[:])
                nc.sync.dma_start(
                    vn_d[tsl, :].rearrange("(tt p) e -> p tt e", p=P), v_sb[:])

                # permute q/k to (wb, h, wi, eo) order for pass-B reads
                hsl = slice(c * 8, (c + 1) * 8)
                for src_sb, dst_d in ((q_sb, qn_d), (k_sb, kn_d)):
                    qp_sb = pPm.tile([P, 8, 8, 8, EO], F32R, tag="qp")
                    for wb in range(8):
                        nc.vector.tensor_copy(
                            qp_sb[:, wb],
                            src_sb[:].rearrange(
                                "p eo (h wb wi) -> p wb h wi eo",
                                h=8, wb=8)[:, wb])
                    nc.sync.dma_start(dst_d[:, :, hsl, :, :], qp_sb[:])

                # row attention for the 8 h's of this chunk, in pairs
                # rout layout [p, wb, h8(chunk-local), wi, ds]
                rout_sb = pQK.tile([P, 8, 8, 8, EO], BF16, tag="rout")
                for pr in range(4):
                    psl = slice(pr * P, (pr + 1) * P)
                    pss = psSc.tile([P, P], F32, tag="sc")
                    for et in range(EO):
                        nc.tensor.matmul(pss[:], q_sb[:, et, psl],
                                         k_sb[:, et, psl],
                                         start=(et == 0), stop=(et == EO - 1))
                    pnT = pSm.tile([P, P], BF16, tag="sm_pnT")
                    nc.vector.memset(pnT[0:64, 64:128], 0)
                    nc.vector.memset(pnT[64:128, 0:64], 0)
                    _softmax_block(nc, pSm,
                                   [pss[0:64, 0:64], pss[64:128, 64:128]],
                                   bm_sb, pnT)
                    psav = psAv.tile([P, 1024], F32, tag="av")
                    for ds_ in range(EO):
                        nc.tensor.matmul(
                            psav[:, ds_ * P:(ds_ + 1) * P],
                            v_sb[:, pr, ds_ * P:(ds_ + 1) * P],
                            pnT[:],
                            start=True, stop=True, skip_group_check=True)
                    for hl in range(2):
                        nc.scalar.copy(
                            rout_sb[:, :, pr * 2 + hl, :, :],
                            psav[:].rearrange(
                                "p (ds hl wb wi) -> p hl wb wi ds",
                                ds=8, hl=2, wb=8)[:, hl])
                nc.sync.dma_start(rc_d[:, :, hsl, :, :], rout_sb[:])

        # ---------------- pass B: col attention + output projection ----------
        with tc.tile_pool(name="wB", bufs=1) as wB, \
             tc.tile_pool(name="pB2", bufs=2) as pB2, \
             tc.tile_pool(name="pBg", bufs=1) as pBg, \
             tc.tile_pool(name="pB1", bufs=2) as pB1, \
             tc.tile_pool(name="pB1s", bufs=2) as pB1s, \
             tc.tile_pool(name="pBq", bufs=2) as pBq, \
             tc.tile_pool(name="pSmB", bufs=3) as pSm, \
             tc.tile_pool(name="psProjB", bufs=2, space="PSUM") as psProj, \
             tc.tile_pool(name="psScB", bufs=2, space="PSUM") as psSc, \
             tc.tile_pool(name="psAvB", bufs=2, space="PSUM") as psAv:
            wo_sb = wB.tile([P, EO, D], BF16)

            vn_v = vn_d[:].rearrange("(h w) e -> h w e", w=64)
            for c in range(NCH):
                tsl = slice(c * CH, (c + 1) * CH)
                wsl = slice(c * 8, (c + 1) * 8)
                # chunk layout: free = (eo, h, wl) -- 8 image columns
                qg_sb = pBg.tile([P, 64, 8, EO], F32R, tag="qg")
                kc_sb = pBg.tile([P, 64, 8, EO], F32R, tag="kc")
                nc.sync.dma_start(qg_sb[:], qn_d[:, c, :, :, :])
                nc.sync.dma_start(kc_sb[:], kn_d[:, c, :, :, :])
                # permute Q to (eo, wl, h) so score lhsT pair slices are
                # single contiguous free dims
                qc_sb = pBq.tile([P, EO, CH], F32R, tag="qc")
                nc.vector.tensor_copy(
                    qc_sb[:].rearrange("p eo (w h) -> p eo w h", w=8),
                    qg_sb[:].rearrange("p h w eo -> p eo w h"))
                kcp_sb = pBq.tile([P, EO, CH], F32R, tag="kcp")
                nc.scalar.copy(
                    kcp_sb[:].rearrange("p eo (w h) -> p eo w h", w=8),
                    kc_sb[:].rearrange("p h w eo -> p eo w h"))
                vcw = pB2.tile([P, 4, D], BF16, tag="vcw")
                for wl in range(8):
                    w_abs = c * 8 + wl
                    nc.sync.dma_start(
                        vcw[(wl % 2) * 64:(wl % 2) * 64 + 64, wl // 2, :],
                        vn_v[:, w_abs, :])
                rc_sb = pB1.tile([P, 64, 8, EO], BF16, tag="rc")
                nc.sync.dma_start(rc_sb[:], rc_d[:, c, :, :, :])
                if c == 0:
                    # wo loads queued behind the first chunk's gathers: the
                    # first out-projection only needs slice 0 ~15us in
                    for et in range(EO):
                        nc.sync.dma_start(wo_sb[:, :, et * P:(et + 1) * P],
                                          wov[:, :, et * P:(et + 1) * P])

                sum_sb = pB1s.tile([P, EO, CH], BF16, tag="sum")
                for pr in range(4):
                    psl = slice(pr * P, (pr + 1) * P)
                    wpr = slice(pr * 2, pr * 2 + 2)
                    pss = psSc.tile([P, P], F32, tag="sc")
                    for et in range(EO):
                        nc.tensor.matmul(pss[:], qc_sb[:, et, psl],
                                         kcp_sb[:, et, psl],
                                         start=(et == 0), stop=(et == EO - 1))
                    pnT = pSm.tile([P, P], BF16, tag="sm_pnT")
                    nc.vector.memset(pnT[0:64, 64:128], 0)
                    nc.vector.memset(pnT[64:128, 0:64], 0)
                    _softmax_block(nc, pSm,
                                   [pss[0:64, 0:64], pss[64:128, 64:128]],
                                   bm_sb, pnT)
                    psav = psAv.tile([P, 1024], F32, tag="av")
                    for ds_ in range(EO):
                        nc.tensor.matmul(
                            psav[:, ds_ * P:(ds_ + 1) * P],
                            vcw[:, pr, ds_ * P:(ds_ + 1) * P],
                            pnT[:],
                            start=True, stop=True, skip_group_check=True)
                    nc.vector.tensor_tensor(
                        sum_sb[:, :, psl].rearrange("p ds (wl h) -> p ds wl h",
                                                    wl=2),
                        psav[:].rearrange("p (ds wl h) -> p ds wl h",
                                          ds=8, wl=2),
                        rc_sb[:, :, wpr, :].rearrange("p h w ds -> p ds w h"),
                        ALU.add)

                outT_sb = pB1s.tile([P, EO, CH], BF16, tag="out")
                for et in range(EO):
                    esl = slice(et * P, (et + 1) * P)
                    pso = psProj.tile([P, CH], F32, tag="po")
                    for dt_ in range(EO):
                        nc.tensor.matmul(pso[:], wo_sb[:, dt_, esl],
                                         sum_sb[:, dt_, :],
                                         start=(dt_ == 0), stop=(dt_ == EO - 1))
                    nc.scalar.add(outT_sb[:, et, :], pso[:],
                                  add=bo_sb[:, et:et + 1])
                nc.sync.dma_start(outv[:, :, tsl], outT_sb[:])

    nc.compile()
    return nc


def _prep_maps(x, Wq, bq, Wk, bk, Wv, bv, Wo, bo, sigma):
    gw = 1.0 / (2.0 * float(sigma[0]) ** 2)
    i = np.arange(64, dtype=np.float32)
    dist = np.square(i[:, None] - i[None, :])
    bm_half = (-gw * dist).astype(np.float32)          # [64, 64]
    bm = np.concatenate([bm_half, bm_half], axis=0)    # [128, 64] both halves

    bf = ml_dtypes.bfloat16
    wqT = np.ascontiguousarray(Wq.T).astype(np.float16)
    wkT = np.ascontiguousarray(Wk.T).astype(np.float16)
    wvT = np.ascontiguousarray(Wv.T).astype(np.float16)
    woT = np.ascontiguousarray(Wo.T).astype(bf)
    # fold bv: softmax rows sum to 1 -> out += 2 * bv @ Wo^T
    bo_eff = bo + 2.0 * (Wo @ bv)
    bqt = np.ascontiguousarray(bq.reshape(EO, P).T)
    bkt = np.ascontiguousarray(bk.reshape(EO, P).T)
    bot = np.ascontiguousarray(bo_eff.astype(np.float32).reshape(EO, P).T)

    in_maps = []
    for b in range(B):
        xT = np.ascontiguousarray(x[b].reshape(HW, D).T).astype(np.float16)
        in_maps.append({
            "xT": xT, "wqT": wqT, "wkT": wkT, "wvT": wvT, "woT": woT,
            "bqt": bqt, "bkt": bkt, "bot": bot, "bm": bm,
        })
    return in_maps


def kernel(x, Wq, bq, Wk, bk, Wv, bv, Wo, bo, sigma, **_ignored):
    x = np.asarray(x, np.float32)
    Wq = np.asarray(Wq, np.float32)
    Wk = np.asarray(Wk, np.float32)
    Wv = np.asarray(Wv, np.float32)
    Wo = np.asarray(Wo, np.float32)
    bq = np.asarray(bq, np.float32)
    bk = np.asarray(bk, np.float32)
    bv = np.asarray(bv, np.float32)
    bo = np.asarray(bo, np.float32)
    sigma = np.asarray(sigma, np.float32)

    if "nc" not in _cache:
        _cache["nc"] = _build()
    nc = _cache["nc"]

    in_maps = _prep_maps(x, Wq, bq, Wk, bk, Wv, bv, Wo, bo, sigma)

    trace = bool(int(os.environ.get("GSA_TRACE", "0")))
    ncore = int(os.environ.get("GSA_CORES", str(B)))
    res = bass_utils.run_bass_kernel_spmd(
        nc, in_maps[:ncore], core_ids=list(range(ncore)),
        trace=trace, trace_cores=[0] if trace else None)
    _cache["last_results"] = res

    out = np.zeros((B, H, W, D), dtype=np.float32)
    for b in range(ncore):
        oT = np.asarray(res.results[b]["outT"], dtype=np.float32)
        out[b] = oT.reshape(D, W, H).transpose(2, 1, 0)
    return out


# revision 6
# speedup vs baseline: 1.1496x; 1.0893x over previous
"""GSA layer (Gaussian-biased axial attention) Trainium2 Bass kernel, v2.

Full inputs in, full output out. Shards batch B=8 across 8 NeuronCores
(data parallel, one image per core). Self-contained: hardcodes shapes.

v2: all-bf16 matmul pipeline (fp32 matmuls are 4 cycles/row on TRN2,
bf16 is 1). PSUM accumulation and softmax stay fp32. bf16 intermediates
in DRAM halve pass-B DMA traffic.

Per-core dataflow (image = 64x64 tokens, D=1024):
  pass A (row):  stream xT chunks (512 tokens = 8 image rows):
                 Q,K proj ([e,t] layout), V proj ([t,e] layout),
                 fused row attention (scores bf16 paired matmuls, softmax
                 fp32 on free axis, DVE block transposes, bf16 AV),
                 writes to DRAM: V natural, row-major Q/K, r_outT.
  pass B (col):  per 8-column chunk: col attention (mirrored),
                 add r_outT, fused output projection -> outT col-order.
Host: transposes/reshapes, folds bv into output bias, unshards output.
"""

import os
import numpy as np
import ml_dtypes

import concourse.bass as bass
import concourse.mybir as mybir
import concourse.tile as tile
from concourse import bacc
from concourse import bass_utils

F32 = mybir.dt.float32
F32R = mybir.dt.float32r
F16 = mybir.dt.float16
BF16 = mybir.dt.bfloat16
AX = mybir.AxisListType
ALU = mybir.AluOpType
ACTF = mybir.ActivationFunctionType

B, H, W, D = 8, 64, 64, 1024
P = 128
HW = H * W            # 4096 tokens per image
CH = 512              # token chunk (8 image rows / 8 image cols)
NCH = HW // CH        # 8 chunks
EO = D // P           # 8 partition tiles of the 1024 dim

_cache = {}


def _softmax_block(nc, pool, half_aps, bm_sb, pnT):
    """Softmax over the free axis of two [64,64] PSUM score blocks
    (half_aps[hl], partition base hl*64), then write transposed bf16
    weights into pnT [128, 64] (half hl at rows hl*64:hl*64+64)."""
    sc = pool.tile([P, 64], F32, tag="sm_sc")
    negm = pool.tile([P, 1], F32, tag="sm_negm")
    ssum = pool.tile([P, 1], F32, tag="sm_ssum")
    rinv = pool.tile([P, 1], F32, tag="sm_rinv")
    pn = pool.tile([P, 64], F32, tag="sm_pn")
    pnn = pool.tile([P, 64], BF16, tag="sm_pnn")
    for hl in range(2):
        blk = slice(hl * 64, hl * 64 + 64)
        nc.vector.tensor_tensor(sc[blk, :], half_aps[hl],
                                bm_sb[blk, :], ALU.add)
        nc.vector.tensor_reduce(negm[blk, :], sc[blk, :], axis=AX.X,
                                op=ALU.max, negate=True)
        nc.scalar.activation(pn[blk, :], sc[blk, :], ACTF.Exp,
                             bias=negm[blk, 0:1], accum_out=ssum[blk, 0:1])
    nc.vector.reciprocal(rinv[:], ssum[:])
    nc.vector.tensor_scalar_mul(pnn[:], pn[:], rinv[:, 0:1])
    # block-diagonal pnT [128, 128]: off-diag zeroed by caller; transpose
    # each 64x64 half into its diagonal block via 4 DVE 32x32 transposes
    for hl in range(2):
        o = hl * 64
        for bi in range(2):
            for bj in range(2):
                nc.vector.transpose(
                    pnT[o + bi * 32:o + bi * 32 + 32,
                        o + bj * 32:o + bj * 32 + 32],
                    pnn[o + bj * 32:o + bj * 32 + 32, bi * 32:bi * 32 + 32])


def _build():
    nc = bacc.Bacc("TRN2", target_bir_lowering=False, debug=False,
                   num_devices=8)

    xT_d = nc.dram_tensor("xT", [D, HW], F16, kind="ExternalInput").ap()
    wq_d = nc.dram_tensor("wqT", [D, D], F16, kind="ExternalInput").ap()
    wk_d = nc.dram_tensor("wkT", [D, D], F16, kind="ExternalInput").ap()
    wv_d = nc.dram_tensor("wvT", [D, D], F16, kind="ExternalInput").ap()
    wo_d = nc.dram_tensor("woT", [D, D], BF16, kind="ExternalInput").ap()
    bq_d = nc.dram_tensor("bqt", [P, EO], F32, kind="ExternalInput").ap()
    bk_d = nc.dram_tensor("bkt", [P, EO], F32, kind="ExternalInput").ap()
    bo_d = nc.dram_tensor("bot", [P, EO], F32, kind="ExternalInput").ap()
    bm_d = nc.dram_tensor("bm", [P, 64], F32, kind="ExternalInput").ap()
    out_d = nc.dram_tensor("outT", [D, HW], BF16, kind="ExternalOutput").ap()

    xTv = xT_d.rearrange("(do p) t -> p do t", p=P)      # [128, 8, 4096]
    wqv = wq_d.rearrange("(do p) e -> p do e", p=P)
    wkv = wk_d.rearrange("(do p) e -> p do e", p=P)
    wvv = wv_d.rearrange("(do p) e -> p do e", p=P)
    wov = wo_d.rearrange("(do p) e -> p do e", p=P)
    outv = out_d.rearrange("(eo p) t -> p eo t", p=P)

    with tile.TileContext(nc) as tc:
      with tc.tile_pool(name="dram", bufs=1, space="DRAM") as dpool, \
           tc.tile_pool(name="consts", bufs=1) as cpool:
        # block-tiled token layout [p, w_blk, h, w_in, eo]: eo innermost
        # makes pass-A writes 2KB-contiguous and pass-B reads 16KB-contiguous
        qn_d = dpool.tile([P, 8, H, 8, EO], F32R)
        kn_d = dpool.tile([P, 8, H, 8, EO], F32R)
        vn_d = dpool.tile([HW, D], BF16)          # V natural row-order
        rc_d = dpool.tile([P, 8, H, 8, EO], BF16)

        bm_sb = cpool.tile([P, 64], F32)
        nc.sync.dma_start(bm_sb[:], bm_d)
        bo_sb = cpool.tile([P, EO], F32)
        nc.sync.dma_start(bo_sb[:], bo_d)

        # ---------------- pass A: projections + row attention ----------------
        with tc.tile_pool(name="wA", bufs=1) as wA, \
             tc.tile_pool(name="pA2", bufs=2) as pA2, \
             tc.tile_pool(name="pPm", bufs=1) as pPm, \
             tc.tile_pool(name="pA1", bufs=2) as pA1, \
             tc.tile_pool(name="pQK", bufs=1) as pQK, \
             tc.tile_pool(name="pSm", bufs=3) as pSm, \
             tc.tile_pool(name="psProj", bufs=2, space="PSUM") as psProj, \
             tc.tile_pool(name="psSc", bufs=2, space="PSUM") as psSc, \
             tc.tile_pool(name="psAv", bufs=2, space="PSUM") as psAv:
            wq_sb = wA.tile([P, EO, D], F16)
            wk_sb = wA.tile([P, EO, D], F16)
            wv_sb = wA.tile([P, EO, D], F16)
            bq_sb = wA.tile([P, EO], F32)
            bk_sb = wA.tile([P, EO], F32)
            x0_sb = pA2.tile([P, EO, CH], F16, tag="x")
            nc.sync.dma_start(x0_sb[:], xTv[:, :, 0:CH])
            nc.sync.dma_start(bq_sb[:], bq_d)
            nc.sync.dma_start(bk_sb[:], bk_d)
            for esl_i in range(EO):
                esl2 = slice(esl_i * P, (esl_i + 1) * P)
                nc.sync.dma_start(wq_sb[:, :, esl2], wqv[:, :, esl2])
                nc.sync.dma_start(wk_sb[:, :, esl2], wkv[:, :, esl2])

            for c in range(NCH):
                tsl = slice(c * CH, (c + 1) * CH)
                if c == 0:
                    x_sb = x0_sb
                    nc.sync.dma_start(wv_sb[:], wvv)
                else:
                    x_sb = pA2.tile([P, EO, CH], F16, tag="x")
                    nc.sync.dma_start(x_sb[:], xTv[:, :, tsl])

                q_sb = pQK.tile([P, EO, CH], F16, tag="q")
                k_sb = pQK.tile([P, EO, CH], F16, tag="k")
                for et in range(EO):
                    esl = slice(et * P, (et + 1) * P)
                    psq = psProj.tile([P, CH], F32, tag="pp")
                    for dt_ in range(EO):
                        nc.tensor.matmul(psq[:], wq_sb[:, dt_, esl],
                                         x_sb[:, dt_, :],
                                         start=(dt_ == 0), stop=(dt_ == EO - 1))
                    nc.scalar.add(q_sb[:, et, :], psq[:], add=bq_sb[:, et:et + 1])
                    psk = psProj.tile([P, CH], F32, tag="pp")
                    for dt_ in range(EO):
                        nc.tensor.matmul(psk[:], wk_sb[:, dt_, esl],
                                         x_sb[:, dt_, :],
                                         start=(dt_ == 0), stop=(dt_ == EO - 1))
                    nc.scalar.add(k_sb[:, et, :], psk[:], add=bk_sb[:, et:et + 1])

                # V natural [t, e] bf16
                v_sb = pQK.tile([P, CH // P, D], BF16, tag="v")
                for tt in range(CH // P):
                    for eh in range(2):
                        psv = psProj.tile([P, 512], F32, tag="pp")
                        for dt_ in range(EO):
                            nc.tensor.matmul(
                                psv[:], x_sb[:, dt_, tt * P:(tt + 1) * P],
                                wv_sb[:, dt_, eh * 512:(eh + 1) * 512],
                                start=(dt_ == 0), stop=(dt_ == EO - 1))
                        nc.scalar.copy(v_sb[:, tt, eh * 512:(eh + 1) * 512], psv[:])
                nc.sync.dma_start(
                    vn_d[tsl, :].rearrange("(tt p) e -> p tt e", p=P), v_sb[:])

                # permute q/k to (wb, h, wi, eo) order for pass-B reads
                hsl = slice(c * 8, (c + 1) * 8)
                for src_sb, dst_d in ((q_sb, qn_d), (k_sb, kn_d)):
                    qp_sb = pPm.tile([P, 8, 8, 8, EO], F32R, tag="qp")
                    for wb in range(8):
                        nc.vector.tensor_copy(
                            qp_sb[:, wb],
                            src_sb[:].rearrange(
                                "p eo (h wb wi) -> p wb h wi eo",
                                h=8, wb=8)[:, wb])
                    nc.sync.dma_start(dst_d[:, :, hsl, :, :], qp_sb[:])

                # row attention for the 8 h's of this chunk, in pairs
                # rout layout [p, wb, h8(chunk-local), wi, ds]
                rout_sb = pQK.tile([P, 8, 8, 8, EO], BF16, tag="rout")
                for pr in range(4):
                    psl = slice(pr * P, (pr + 1) * P)
                    pss = psSc.tile([P, P], F32, tag="sc")
                    for et in range(EO):
                        nc.tensor.matmul(pss[:], q_sb[:, et, psl],
                                         k_sb[:, et, psl],
                                         start=(et == 0), stop=(et == EO - 1))
                    pnT = pSm.tile([P, P], BF16, tag="sm_pnT")
                    nc.vector.memset(pnT[0:64, 64:128], 0)
                    nc.vector.memset(pnT[64:128, 0:64], 0)
                    _softmax_block(nc, pSm,
                                   [pss[0:64, 0:64], pss[64:128, 64:128]],
                                   bm_sb, pnT)
                    psav = psAv.tile([P, 1024], F32, tag="av")
                    for ds_ in range(EO):
                        nc.tensor.matmul(
                            psav[:, ds_ * P:(ds_ + 1) * P],
                            v_sb[:, pr, ds_ * P:(ds_ + 1) * P],
                            pnT[:],
                            start=True, stop=True, skip_group_check=True)
                    for hl in range(2):
                        nc.scalar.copy(
                            rout_sb[:, :, pr * 2 + hl, :, :],
                            psav[:].rearrange(
                                "p (ds hl wb wi) -> p hl wb wi ds",
                                ds=8, hl=2, wb=8)[:, hl])
                nc.sync.dma_start(rc_d[:, :, hsl, :, :], rout_sb[:])

        # ---------------- pass B: col attention + output projection ----------
        with tc.tile_pool(name="wB", bufs=1) as wB, \
             tc.tile_pool(name="pB2", bufs=2) as pB2, \
             tc.tile_pool(name="pBg", bufs=1) as pBg, \
             tc.tile_pool(name="pB1", bufs=2) as pB1, \
             tc.tile_pool(name="pB1s", bufs=2) as pB1s, \
             tc.tile_pool(name="pBq", bufs=2) as pBq, \
             tc.tile_pool(name="pSmB", bufs=3) as pSm, \
             tc.tile_pool(name="psProjB", bufs=2, space="PSUM") as psProj, \
             tc.tile_pool(name="psScB", bufs=2, space="PSUM") as psSc, \
             tc.tile_pool(name="psAvB", bufs=2, space="PSUM") as psAv:
            wo_sb = wB.tile([P, EO, D], BF16)

            vn_v = vn_d[:].rearrange("(h w) e -> h w e", w=64)
            for c in range(NCH):
                tsl = slice(c * CH, (c + 1) * CH)
                wsl = slice(c * 8, (c + 1) * 8)
                # chunk layout: free = (eo, h, wl) -- 8 image columns
                qg_sb = pBg.tile([P, 64, 8, EO], F32R, tag="qg")
                kc_sb = pBg.tile([P, 64, 8, EO], F32R, tag="kc")
                nc.sync.dma_start(qg_sb[:], qn_d[:, c, :, :, :])
                nc.sync.dma_start(kc_sb[:], kn_d[:, c, :, :, :])
                # permute Q to (eo, wl, h) so score lhsT pair slices are
                # single contiguous free dims
                qc_sb = pBq.tile([P, EO, CH], F16, tag="qc")
                nc.vector.tensor_copy(
                    qc_sb[:].rearrange("p eo (w h) -> p eo w h", w=8),
                    qg_sb[:].rearrange("p h w eo -> p eo w h"))
                kcp_sb = pBq.tile([P, EO, CH], F16, tag="kcp")
                nc.scalar.copy(
                    kcp_sb[:].rearrange("p eo (w h) -> p eo w h", w=8),
                    kc_sb[:].rearrange("p h w eo -> p eo w h"))
                vcw = pB2.tile([P, 4, D], BF16, tag="vcw")
                for wl in range(8):
                    w_abs = c * 8 + wl
                    nc.sync.dma_start(
                        vcw[(wl % 2) * 64:(wl % 2) * 64 + 64, wl // 2, :],
                        vn_v[:, w_abs, :])
                rc_sb = pB1.tile([P, 64, 8, EO], BF16, tag="rc")
                nc.sync.dma_start(rc_sb[:], rc_d[:, c, :, :, :])
                if c == 0:
                    # wo loads queued behind the first chunk's gathers: the
                    # first out-projection only needs slice 0 ~15us in
                    for et in range(EO):
                        nc.sync.dma_start(wo_sb[:, :, et * P:(et + 1) * P],
                                          wov[:, :, et * P:(et + 1) * P])

                sum_sb = pB1s.tile([P, EO, CH], BF16, tag="sum")
                for pr in range(4):
                    psl = slice(pr * P, (pr + 1) * P)
                    wpr = slice(pr * 2, pr * 2 + 2)
                    pss = psSc.tile([P, P], F32, tag="sc")
                    for et in range(EO):
                        nc.tensor.matmul(pss[:], qc_sb[:, et, psl],
                                         kcp_sb[:, et, psl],
                                         start=(et == 0), stop=(et == EO - 1))
                    pnT = pSm.tile([P, P], BF16, tag="sm_pnT")
                    nc.vector.memset(pnT[0:64, 64:128], 0)
                    nc.vector.memset(pnT[64:128, 0:64], 0)
                    _softmax_block(nc, pSm,
                                   [pss[0:64, 0:64], pss[64:128, 64:128]],
                                   bm_sb, pnT)
                    psav = psAv.tile([P, 1024], F32, tag="av")
                    for ds_ in range(EO):
                        nc.tensor.matmul(
                            psav[:, ds_ * P:(ds_ + 1) * P],
                            vcw[:, pr, ds_ * P:(ds_ + 1) * P],
                            pnT[:],
                            start=True, stop=True, skip_group_check=True)
                    nc.vector.tensor_tensor(
                        sum_sb[:, :, psl].rearrange("p ds (wl h) -> p ds wl h",
                                                    wl=2),
                        psav[:].rearrange("p (ds wl h) -> p ds wl h",
                                          ds=8, wl=2),
                        rc_sb[:, :, wpr, :].rearrange("p h w ds -> p ds w h"),
                        ALU.add)

                outT_sb = pB1s.tile([P, EO, CH], BF16, tag="out")
                for et in range(EO):
                    esl = slice(et * P, (et + 1) * P)
                    pso = psProj.tile([P, CH], F32, tag="po")
                    for dt_ in range(EO):
                        nc.tensor.matmul(pso[:], wo_sb[:, dt_, esl],
                                         sum_sb[:, dt_, :],
                                         start=(dt_ == 0), stop=(dt_ == EO - 1))
                    nc.scalar.add(outT_sb[:, et, :], pso[:],
                                  add=bo_sb[:, et:et + 1])
                nc.sync.dma_start(outv[:, :, tsl], outT_sb[:])

    nc.compile()
    return nc


def _prep_maps(x, Wq, bq, Wk, bk, Wv, bv, Wo, bo, sigma):
    gw = 1.0 / (2.0 * float(sigma[0]) ** 2)
    i = np.arange(64, dtype=np.float32)
    dist = np.square(i[:, None] - i[None, :])
    bm_half = (-gw * dist).astype(np.float32)          # [64, 64]
    bm = np.concatenate([bm_half, bm_half], axis=0)    # [128, 64] both halves

    bf = ml_dtypes.bfloat16
    wqT = np.ascontiguousarray(Wq.T).astype(np.float16)
    wkT = np.ascontiguousarray(Wk.T).astype(np.float16)
    wvT = np.ascontiguousarray(Wv.T).astype(np.float16)
    woT = np.ascontiguousarray(Wo.T).astype(bf)
    # fold bv: softmax rows sum to 1 -> out += 2 * bv @ Wo^T
    bo_eff = bo + 2.0 * (Wo @ bv)
    bqt = np.ascontiguousarray(bq.reshape(EO, P).T)
    bkt = np.ascontiguousarray(bk.reshape(EO, P).T)
    bot = np.ascontiguousarray(bo_eff.astype(np.float32).reshape(EO, P).T)

    in_maps = []
    for b in range(B):
        xT = np.ascontiguousarray(x[b].reshape(HW, D).T).astype(np.float16)
        in_maps.append({
            "xT": xT, "wqT": wqT, "wkT": wkT, "wvT": wvT, "woT": woT,
            "bqt": bqt, "bkt": bkt, "bot": bot, "bm": bm,
        })
    return in_maps


def kernel(x, Wq, bq, Wk, bk, Wv, bv, Wo, bo, sigma, **_ignored):
    x = np.asarray(x, np.float32)
    Wq = np.asarray(Wq, np.float32)
    Wk = np.asarray(Wk, np.float32)
    Wv = np.asarray(Wv, np.float32)
    Wo = np.asarray(Wo, np.float32)
    bq = np.asarray(bq, np.float32)
    bk = np.asarray(bk, np.float32)
    bv = np.asarray(bv, np.float32)
    bo = np.asarray(bo, np.float32)
    sigma = np.asarray(sigma, np.float32)

    if "nc" not in _cache:
        _cache["nc"] = _build()
    nc = _cache["nc"]

    in_maps = _prep_maps(x, Wq, bq, Wk, bk, Wv, bv, Wo, bo, sigma)

    trace = bool(int(os.environ.get("GSA_TRACE", "0")))
    ncore = int(os.environ.get("GSA_CORES", str(B)))
    res = bass_utils.run_bass_kernel_spmd(
        nc, in_maps[:ncore], core_ids=list(range(ncore)),
        trace=trace, trace_cores=[0] if trace else None)
    _cache["last_results"] = res

    out = np.zeros((B, H, W, D), dtype=np.float32)
    for b in range(ncore):
        oT = np.asarray(res.results[b]["outT"], dtype=np.float32)
        out[b] = oT.reshape(D, W, H).transpose(2, 1, 0)
    return out


# revision 7
# speedup vs baseline: 1.1538x; 1.0037x over previous
"""GSA layer (Gaussian-biased axial attention) Trainium2 Bass kernel, v2.

Full inputs in, full output out. Shards batch B=8 across 8 NeuronCores
(data parallel, one image per core). Self-contained: hardcodes shapes.

v2: all-bf16 matmul pipeline (fp32 matmuls are 4 cycles/row on TRN2,
bf16 is 1). PSUM accumulation and softmax stay fp32. bf16 intermediates
in DRAM halve pass-B DMA traffic.

Per-core dataflow (image = 64x64 tokens, D=1024):
  pass A (row):  stream xT chunks (512 tokens = 8 image rows):
                 Q,K proj ([e,t] layout), V proj ([t,e] layout),
                 fused row attention (scores bf16 paired matmuls, softmax
                 fp32 on free axis, DVE block transposes, bf16 AV),
                 writes to DRAM: V natural, row-major Q/K, r_outT.
  pass B (col):  per 8-column chunk: col attention (mirrored),
                 add r_outT, fused output projection -> outT col-order.
Host: transposes/reshapes, folds bv into output bias, unshards output.
"""

import os
import numpy as np
import ml_dtypes

import concourse.bass as bass
import concourse.mybir as mybir
import concourse.tile as tile
from concourse import bacc
from concourse import bass_utils

F32 = mybir.dt.float32
F32R = mybir.dt.float32r
F16 = mybir.dt.float16
BF16 = mybir.dt.bfloat16
AX = mybir.AxisListType
ALU = mybir.AluOpType
ACTF = mybir.ActivationFunctionType

B, H, W, D = 8, 64, 64, 1024
P = 128
HW = H * W            # 4096 tokens per image
CH = 512              # token chunk (8 image rows / 8 image cols)
NCH = HW // CH        # 8 chunks
EO = D // P           # 8 partition tiles of the 1024 dim

_cache = {}


def _softmax_block(nc, pool, half_aps, bm_sb, pnT):
    """Softmax over the free axis of two [64,64] PSUM score blocks
    (half_aps[hl], partition base hl*64), then write transposed bf16
    weights into pnT [128, 64] (half hl at rows hl*64:hl*64+64)."""
    sc = pool.tile([P, 64], F32, tag="sm_sc")
    negm = pool.tile([P, 1], F32, tag="sm_negm")
    ssum = pool.tile([P, 1], F32, tag="sm_ssum")
    rinv = pool.tile([P, 1], F32, tag="sm_rinv")
    pn = pool.tile([P, 64], F32, tag="sm_pn")
    pnn = pool.tile([P, 64], BF16, tag="sm_pnn")
    for hl in range(2):
        blk = slice(hl * 64, hl * 64 + 64)
        nc.vector.tensor_tensor(sc[blk, :], half_aps[hl],
                                bm_sb[blk, :], ALU.add)
        nc.vector.tensor_reduce(negm[blk, :], sc[blk, :], axis=AX.X,
                                op=ALU.max, negate=True)
        nc.scalar.activation(pn[blk, :], sc[blk, :], ACTF.Exp,
                             bias=negm[blk, 0:1], accum_out=ssum[blk, 0:1])
    nc.vector.reciprocal(rinv[:], ssum[:])
    nc.vector.tensor_scalar_mul(pnn[:], pn[:], rinv[:, 0:1])
    # block-diagonal pnT [128, 128]: off-diag zeroed by caller; transpose
    # each 64x64 half into its diagonal block via 4 DVE 32x32 transposes
    for hl in range(2):
        o = hl * 64
        for bi in range(2):
            for bj in range(2):
                nc.vector.transpose(
                    pnT[o + bi * 32:o + bi * 32 + 32,
                        o + bj * 32:o + bj * 32 + 32],
                    pnn[o + bj * 32:o + bj * 32 + 32, bi * 32:bi * 32 + 32])


def _build():
    nc = bacc.Bacc("TRN2", target_bir_lowering=False, debug=False,
                   num_devices=8)

    xT_d = nc.dram_tensor("xT", [D, HW], F16, kind="ExternalInput").ap()
    wq_d = nc.dram_tensor("wqT", [D, D], F16, kind="ExternalInput").ap()
    wk_d = nc.dram_tensor("wkT", [D, D], F16, kind="ExternalInput").ap()
    wv_d = nc.dram_tensor("wvT", [D, D], F16, kind="ExternalInput").ap()
    wo_d = nc.dram_tensor("woT", [D, D], BF16, kind="ExternalInput").ap()
    bq_d = nc.dram_tensor("bqt", [P, EO], F32, kind="ExternalInput").ap()
    bk_d = nc.dram_tensor("bkt", [P, EO], F32, kind="ExternalInput").ap()
    bo_d = nc.dram_tensor("bot", [P, EO], F32, kind="ExternalInput").ap()
    bm_d = nc.dram_tensor("bm", [P, 64], F32, kind="ExternalInput").ap()
    out_d = nc.dram_tensor("outT", [D, HW], BF16, kind="ExternalOutput").ap()

    xTv = xT_d.rearrange("(do p) t -> p do t", p=P)      # [128, 8, 4096]
    wqv = wq_d.rearrange("(do p) e -> p do e", p=P)
    wkv = wk_d.rearrange("(do p) e -> p do e", p=P)
    wvv = wv_d.rearrange("(do p) e -> p do e", p=P)
    wov = wo_d.rearrange("(do p) e -> p do e", p=P)
    outv = out_d.rearrange("(eo p) t -> p eo t", p=P)

    with tile.TileContext(nc) as tc:
      with tc.tile_pool(name="dram", bufs=1, space="DRAM") as dpool, \
           tc.tile_pool(name="consts", bufs=1) as cpool:
        # block-tiled token layout [p, w_blk, h, w_in, eo]: eo innermost
        # makes pass-A writes 2KB-contiguous and pass-B reads 16KB-contiguous
        qn_d = dpool.tile([P, 8, H, 8, EO], F16)
        kn_d = dpool.tile([P, 8, H, 8, EO], F16)
        vn_d = dpool.tile([HW, D], BF16)          # V natural row-order
        rc_d = dpool.tile([P, 8, H, 8, EO], BF16)

        bm_sb = cpool.tile([P, 64], F32)
        nc.sync.dma_start(bm_sb[:], bm_d)
        bo_sb = cpool.tile([P, EO], F32)
        nc.sync.dma_start(bo_sb[:], bo_d)

        # ---------------- pass A: projections + row attention ----------------
        with tc.tile_pool(name="wA", bufs=1) as wA, \
             tc.tile_pool(name="pA2", bufs=2) as pA2, \
             tc.tile_pool(name="pPm", bufs=1) as pPm, \
             tc.tile_pool(name="pA1", bufs=2) as pA1, \
             tc.tile_pool(name="pQK", bufs=1) as pQK, \
             tc.tile_pool(name="pSm", bufs=3) as pSm, \
             tc.tile_pool(name="psProj", bufs=2, space="PSUM") as psProj, \
             tc.tile_pool(name="psSc", bufs=2, space="PSUM") as psSc, \
             tc.tile_pool(name="psAv", bufs=2, space="PSUM") as psAv:
            wq_sb = wA.tile([P, EO, D], F16)
            wk_sb = wA.tile([P, EO, D], F16)
            wv_sb = wA.tile([P, EO, D], F16)
            bq_sb = wA.tile([P, EO], F32)
            bk_sb = wA.tile([P, EO], F32)
            x0_sb = pA2.tile([P, EO, CH], F16, tag="x")
            nc.sync.dma_start(x0_sb[:], xTv[:, :, 0:CH])
            nc.sync.dma_start(bq_sb[:], bq_d)
            nc.sync.dma_start(bk_sb[:], bk_d)
            for esl_i in range(EO):
                esl2 = slice(esl_i * P, (esl_i + 1) * P)
                nc.sync.dma_start(wq_sb[:, :, esl2], wqv[:, :, esl2])
                nc.sync.dma_start(wk_sb[:, :, esl2], wkv[:, :, esl2])

            for c in range(NCH):
                tsl = slice(c * CH, (c + 1) * CH)
                if c == 0:
                    x_sb = x0_sb
                    nc.sync.dma_start(wv_sb[:], wvv)
                else:
                    x_sb = pA2.tile([P, EO, CH], F16, tag="x")
                    nc.sync.dma_start(x_sb[:], xTv[:, :, tsl])

                q_sb = pQK.tile([P, EO, CH], F16, tag="q")
                k_sb = pQK.tile([P, EO, CH], F16, tag="k")
                for et in range(EO):
                    esl = slice(et * P, (et + 1) * P)
                    psq = psProj.tile([P, CH], F32, tag="pp")
                    for dt_ in range(EO):
                        nc.tensor.matmul(psq[:], wq_sb[:, dt_, esl],
                                         x_sb[:, dt_, :],
                                         start=(dt_ == 0), stop=(dt_ == EO - 1))
                    nc.scalar.add(q_sb[:, et, :], psq[:], add=bq_sb[:, et:et + 1])
                    psk = psProj.tile([P, CH], F32, tag="pp")
                    for dt_ in range(EO):
                        nc.tensor.matmul(psk[:], wk_sb[:, dt_, esl],
                                         x_sb[:, dt_, :],
                                         start=(dt_ == 0), stop=(dt_ == EO - 1))
                    nc.scalar.add(k_sb[:, et, :], psk[:], add=bk_sb[:, et:et + 1])

                # V natural [t, e] bf16
                v_sb = pQK.tile([P, CH // P, D], BF16, tag="v")
                for tt in range(CH // P):
                    for eh in range(2):
                        psv = psProj.tile([P, 512], F32, tag="pp")
                        for dt_ in range(EO):
                            nc.tensor.matmul(
                                psv[:], x_sb[:, dt_, tt * P:(tt + 1) * P],
                                wv_sb[:, dt_, eh * 512:(eh + 1) * 512],
                                start=(dt_ == 0), stop=(dt_ == EO - 1))
                        nc.scalar.copy(v_sb[:, tt, eh * 512:(eh + 1) * 512], psv[:])
                nc.sync.dma_start(
                    vn_d[tsl, :].rearrange("(tt p) e -> p tt e", p=P), v_sb[:])

                # permute q/k to (wb, h, wi, eo) order for pass-B reads
                hsl = slice(c * 8, (c + 1) * 8)
                for src_sb, dst_d in ((q_sb, qn_d), (k_sb, kn_d)):
                    qp_sb = pPm.tile([P, 8, 8, 8, EO], F16, tag="qp")
                    for wb in range(8):
                        nc.vector.tensor_copy(
                            qp_sb[:, wb],
                            src_sb[:].rearrange(
                                "p eo (h wb wi) -> p wb h wi eo",
                                h=8, wb=8)[:, wb])
                    nc.sync.dma_start(dst_d[:, :, hsl, :, :], qp_sb[:])

                # row attention for the 8 h's of this chunk, in pairs
                # rout layout [p, wb, h8(chunk-local), wi, ds]
                rout_sb = pQK.tile([P, 8, 8, 8, EO], BF16, tag="rout")
                for pr in range(4):
                    psl = slice(pr * P, (pr + 1) * P)
                    pss = psSc.tile([P, P], F32, tag="sc")
                    for et in range(EO):
                        nc.tensor.matmul(pss[:], q_sb[:, et, psl],
                                         k_sb[:, et, psl],
                                         start=(et == 0), stop=(et == EO - 1))
                    pnT = pSm.tile([P, P], BF16, tag="sm_pnT")
                    nc.vector.memset(pnT[0:64, 64:128], 0)
                    nc.vector.memset(pnT[64:128, 0:64], 0)
                    _softmax_block(nc, pSm,
                                   [pss[0:64, 0:64], pss[64:128, 64:128]],
                                   bm_sb, pnT)
                    psav = psAv.tile([P, 1024], F32, tag="av")
                    for ds_ in range(EO):
                        nc.tensor.matmul(
                            psav[:, ds_ * P:(ds_ + 1) * P],
                            v_sb[:, pr, ds_ * P:(ds_ + 1) * P],
                            pnT[:],
                            start=True, stop=True, skip_group_check=True)
                    for hl in range(2):
                        nc.scalar.copy(
                            rout_sb[:, :, pr * 2 + hl, :, :],
                            psav[:].rearrange(
                                "p (ds hl wb wi) -> p hl wb wi ds",
                                ds=8, hl=2, wb=8)[:, hl])
                nc.sync.dma_start(rc_d[:, :, hsl, :, :], rout_sb[:])

        # ---------------- pass B: col attention + output projection ----------
        with tc.tile_pool(name="wB", bufs=1) as wB, \
             tc.tile_pool(name="pB2", bufs=2) as pB2, \
             tc.tile_pool(name="pBg", bufs=1) as pBg, \
             tc.tile_pool(name="pB1", bufs=2) as pB1, \
             tc.tile_pool(name="pB1s", bufs=2) as pB1s, \
             tc.tile_pool(name="pBq", bufs=2) as pBq, \
             tc.tile_pool(name="pSmB", bufs=3) as pSm, \
             tc.tile_pool(name="psProjB", bufs=2, space="PSUM") as psProj, \
             tc.tile_pool(name="psScB", bufs=2, space="PSUM") as psSc, \
             tc.tile_pool(name="psAvB", bufs=2, space="PSUM") as psAv:
            wo_sb = wB.tile([P, EO, D], BF16)

            vn_v = vn_d[:].rearrange("(h w) e -> h w e", w=64)
            for c in range(NCH):
                tsl = slice(c * CH, (c + 1) * CH)
                wsl = slice(c * 8, (c + 1) * 8)
                # chunk layout: free = (eo, h, wl) -- 8 image columns
                qg_sb = pBg.tile([P, 64, 8, EO], F16, tag="qg")
                kc_sb = pBg.tile([P, 64, 8, EO], F16, tag="kc")
                nc.sync.dma_start(qg_sb[:], qn_d[:, c, :, :, :])
                nc.sync.dma_start(kc_sb[:], kn_d[:, c, :, :, :])
                # permute Q to (eo, wl, h) so score lhsT pair slices are
                # single contiguous free dims
                qc_sb = pBq.tile([P, EO, CH], F16, tag="qc")
                nc.vector.tensor_copy(
                    qc_sb[:].rearrange("p eo (w h) -> p eo w h", w=8),
                    qg_sb[:].rearrange("p h w eo -> p eo w h"))
                kcp_sb = pBq.tile([P, EO, CH], F16, tag="kcp")
                nc.scalar.copy(
                    kcp_sb[:].rearrange("p eo (w h) -> p eo w h", w=8),
                    kc_sb[:].rearrange("p h w eo -> p eo w h"))
                vcw = pB2.tile([P, 4, D], BF16, tag="vcw")
                for wl in range(8):
                    w_abs = c * 8 + wl
                    nc.sync.dma_start(
                        vcw[(wl % 2) * 64:(wl % 2) * 64 + 64, wl // 2, :],
                        vn_v[:, w_abs, :])
                rc_sb = pB1.tile([P, 64, 8, EO], BF16, tag="rc")
                nc.sync.dma_start(rc_sb[:], rc_d[:, c, :, :, :])
                if c == 0:
                    # wo loads queued behind the first chunk's gathers: the
                    # first out-projection only needs slice 0 ~15us in
                    for et in range(EO):
                        nc.sync.dma_start(wo_sb[:, :, et * P:(et + 1) * P],
                                          wov[:, :, et * P:(et + 1) * P])

                sum_sb = pB1s.tile([P, EO, CH], BF16, tag="sum")
                for pr in range(4):
                    psl = slice(pr * P, (pr + 1) * P)
                    wpr = slice(pr * 2, pr * 2 + 2)
                    pss = psSc.tile([P, P], F32, tag="sc")
                    for et in range(EO):
                        nc.tensor.matmul(pss[:], qc_sb[:, et, psl],
                                         kcp_sb[:, et, psl],
                                         start=(et == 0), stop=(et == EO - 1))
                    pnT = pSm.tile([P, P], BF16, tag="sm_pnT")
                    nc.vector.memset(pnT[0:64, 64:128], 0)
                    nc.vector.memset(pnT[64:128, 0:64], 0)
                    _softmax_block(nc, pSm,
                                   [pss[0:64, 0:64], pss[64:128, 64:128]],
                                   bm_sb, pnT)
                    psav = psAv.tile([P, 1024], F32, tag="av")
                    for ds_ in range(EO):
                        nc.tensor.matmul(
                            psav[:, ds_ * P:(ds_ + 1) * P],
                            vcw[:, pr, ds_ * P:(ds_ + 1) * P],
                            pnT[:],
                            start=True, stop=True, skip_group_check=True)
                    nc.vector.tensor_tensor(
                        sum_sb[:, :, psl].rearrange("p ds (wl h) -> p ds wl h",
                                                    wl=2),
                        psav[:].rearrange("p (ds wl h) -> p ds wl h",
                                          ds=8, wl=2),
                        rc_sb[:, :, wpr, :].rearrange("p h w ds -> p ds w h"),
                        ALU.add)

                outT_sb = pB1s.tile([P, EO, CH], BF16, tag="out")
                for et in range(EO):
                    esl = slice(et * P, (et + 1) * P)
                    pso = psProj.tile([P, CH], F32, tag="po")
                    for dt_ in range(EO):
                        nc.tensor.matmul(pso[:], wo_sb[:, dt_, esl],
                                         sum_sb[:, dt_, :],
                                         start=(dt_ == 0), stop=(dt_ == EO - 1))
                    nc.scalar.add(outT_sb[:, et, :], pso[:],
                                  add=bo_sb[:, et:et + 1])
                nc.sync.dma_start(outv[:, :, tsl], outT_sb[:])

    nc.compile()
    return nc


def _prep_maps(x, Wq, bq, Wk, bk, Wv, bv, Wo, bo, sigma):
    gw = 1.0 / (2.0 * float(sigma[0]) ** 2)
    i = np.arange(64, dtype=np.float32)
    dist = np.square(i[:, None] - i[None, :])
    bm_half = (-gw * dist).astype(np.float32)          # [64, 64]
    bm = np.concatenate([bm_half, bm_half], axis=0)    # [128, 64] both halves

    bf = ml_dtypes.bfloat16
    wqT = np.ascontiguousarray(Wq.T).astype(np.float16)
    wkT = np.ascontiguousarray(Wk.T).astype(np.float16)
    wvT = np.ascontiguousarray(Wv.T).astype(np.float16)
    woT = np.ascontiguousarray(Wo.T).astype(bf)
    # fold bv: softmax rows sum to 1 -> out += 2 * bv @ Wo^T
    bo_eff = bo + 2.0 * (Wo @ bv)
    bqt = np.ascontiguousarray(bq.reshape(EO, P).T)
    bkt = np.ascontiguousarray(bk.reshape(EO, P).T)
    bot = np.ascontiguousarray(bo_eff.astype(np.float32).reshape(EO, P).T)

    in_maps = []
    for b in range(B):
        xT = np.ascontiguousarray(x[b].reshape(HW, D).T).astype(np.float16)
        in_maps.append({
            "xT": xT, "wqT": wqT, "wkT": wkT, "wvT": wvT, "woT": woT,
            "bqt": bqt, "bkt": bkt, "bot": bot, "bm": bm,
        })
    return in_maps


def kernel(x, Wq, bq, Wk, bk, Wv, bv, Wo, bo, sigma, **_ignored):
    x = np.asarray(x, np.float32)
    Wq = np.asarray(Wq, np.float32)
    Wk = np.asarray(Wk, np.float32)
    Wv = np.asarray(Wv, np.float32)
    Wo = np.asarray(Wo, np.float32)
    bq = np.asarray(bq, np.float32)
    bk = np.asarray(bk, np.float32)
    bv = np.asarray(bv, np.float32)
    bo = np.asarray(bo, np.float32)
    sigma = np.asarray(sigma, np.float32)

    if "nc" not in _cache:
        _cache["nc"] = _build()
    nc = _cache["nc"]

    in_maps = _prep_maps(x, Wq, bq, Wk, bk, Wv, bv, Wo, bo, sigma)

    trace = bool(int(os.environ.get("GSA_TRACE", "0")))
    ncore = int(os.environ.get("GSA_CORES", str(B)))
    res = bass_utils.run_bass_kernel_spmd(
        nc, in_maps[:ncore], core_ids=list(range(ncore)),
        trace=trace, trace_cores=[0] if trace else None)
    _cache["last_results"] = res

    out = np.zeros((B, H, W, D), dtype=np.float32)
    for b in range(ncore):
        oT = np.asarray(res.results[b]["outT"], dtype=np.float32)
        out[b] = oT.reshape(D, W, H).transpose(2, 1, 0)
    return out
